# revision 64
# baseline (speedup 1.0000x reference)
"""GCN (3-layer GCNConv + global mean pool) on 8 Trainium2 NeuronCores.

Math: with S = adjacency+self-loops and D = diag(1/sqrt(deg)),
    conv(h) = relu(D S D h W + b)
and the diagonal scalings commute with the dense W, so each layer is an
UNWEIGHTED gather-sum of pre-scaled rows plus a dense matmul.  The final
conv + mean-pool collapse into a dense host-precomputed pooling matrix:
    out = (Mhat @ h2) @ W2 + b2,   Mhat = diag(1/cnt) S_pool A_norm.

Sharding: nodes dst-partitioned across 8 cores; layer 0 is recomputed on
each core's halo (src nodes of its incident edges) so cores never
communicate.  Optimizations over the plain halo design:
  * per-core xh holds only the deduplicated layer-0 edge sources (3
    int16-index windows, placed by local out-degree), so gather columns
    carry far less padding than a shared node-id-ordered table;
  * self-loop contributions stream in as a dense pre-transposed
    [feat, slot] table (bytes-bound DMA) and enter PSUM via one
    identity matmul per tile instead of per-slot gather descriptors;
  * slot schedules are budgeted per window-count pattern across cores so
    all 8 cores share one compiled program with an exact column profile;
  * PSUM aggregation packs 4 tiles per bank, copied out in one DVE op;
  * h2 never round-trips DRAM: layer-1 outputs stay in SBUF and feed the
    pooling matmuls directly, with the pooling matrix staged in a
    partition-tiled layout (large DMA descriptors).
"""

import hashlib

import numpy as np
import ml_dtypes

P = 128
NCORES = 8
WSTR = 32768      # physical window stride (rows); row w*WSTR is all-zero
WIN0 = 32767      # usable rows per layer-0 source window
WIN1 = 32256      # usable slots per layer-1 source window (512-aligned)
GCOLS = 96        # gather column budget per tile-group (layer 1)
CCAP = 32         # max columns per dma_gather call (layer 1)
GCOLS0 = 64       # layer-0 columns are 512B pair-reads: smaller groups
CCAP0 = 16

BF16 = ml_dtypes.bfloat16
FP8 = ml_dtypes.float8_e4m3


def _h1row(s):
    """Slot -> h1h row (grouped so that one partition's 4 rows of a write
    block are contiguous: 1KB fp8 descriptors).  WIN1 % 512 == 0 keeps
    512-slot blocks window-aligned."""
    w = s // WIN1
    loc = s - w * WIN1
    b = loc // 512
    p = s % P
    j = (s // P) % 4
    return w * WSTR + 1 + b * 512 + p * 4 + j


# ---------------------------------------------------------------------------
# Shared schedule derivation (host + builder + emulator all use this)
# ---------------------------------------------------------------------------

def _schedule(D, NW, gcols=GCOLS, ccap=CCAP):
    """D: [T][NW] column counts.  Returns (groups, calls, colbase, Ctot).

    groups: list of (t0, t1) tile ranges with total columns <= gcols.
    calls: list of (w, ncols, col_off) in global column order; a call's
        columns are consecutive.  Global column order: per group, per
        window (ascending), per tile (ascending), per j.
    colbase: [T][NW] global column offset of (t, w)'s first column.
    """
    T = len(D)
    groups = []
    t = 0
    while t < T:
        # taper: the last tiles get half-size groups so the compute tail
        # after the final gather stays short
        lim = gcols if t < T - 24 else max(gcols // 4, 8)
        tot = sum(D[t])
        t1 = t + 1
        while t1 < T and tot + sum(D[t1]) <= lim:
            tot += sum(D[t1])
            t1 += 1
        groups.append((t, t1))
        t = t1
    colbase = [[0] * NW for _ in range(T)]
    calls = []
    off = 0
    for (t0, t1) in groups:
        for w in range(NW):
            cols = 0
            for tt in range(t0, t1):
                colbase[tt][w] = off + cols
                cols += D[tt][w]
            c0 = 0
            while c0 < cols:
                n = min(ccap, cols - c0)
                calls.append((w, n, off + c0))
                c0 += n
            off += cols
    return groups, calls, colbase, off


def _call_of_col(calls):
    """Map global column -> (call_id, local_col)."""
    m = {}
    for ci, (w, n, off) in enumerate(calls):
        for j in range(n):
            m[off + j] = (ci, j)
    return m


# ---------------------------------------------------------------------------
# Host preprocessing
# ---------------------------------------------------------------------------

def _budget_slots(pats_per_core, round_slots):
    """Shared pattern budget: every distinct window-count vector gets
    max-over-cores slots at a FIXED (lexicographically ordered) position.
    Returns (slotpat [T*P, NW], class ranges dict pat->slice, T)."""
    cat = np.concatenate(pats_per_core)
    upat, inv = np.unique(cat, axis=0, return_inverse=True)
    npat = len(upat)
    counts = np.zeros((len(pats_per_core), npat), np.int64)
    off = 0
    for c, p in enumerate(pats_per_core):
        counts[c] = np.bincount(inv[off:off + len(p)], minlength=npat)
        off += len(p)
    budget = counts.max(axis=0)
    lex = np.lexsort(upat.T[::-1])
    tot = int(budget.sum())
    T = -(-tot // round_slots) * (round_slots // P)
    slotpat = np.zeros((T * P, upat.shape[1]), np.int64)
    starts = np.zeros(npat, np.int64)
    pos = T * P - tot          # pads first
    for pi in lex:
        slotpat[pos:pos + budget[pi]] = upat[pi]
        starts[pi] = pos
        pos += budget[pi]
    return slotpat, upat, starts, T


def _fill_idx(slots_of_edges, rows_of_edges, w_of_edges, jj, colbase, D,
              Ctot, NW):
    """Build the flat int16 gather index list [Ctot*128] (0 = window zero
    row), then wrap for dma_gather: [128, Ctot*8].

    slots_of_edges: slot id per edge; rows_of_edges: +1-based row within
    window; w_of_edges: window id; jj: stable per-(slot,window) rank.
    """
    flat = np.zeros(Ctot * P, np.int16)
    tt = slots_of_edges // P
    pp = slots_of_edges % P
    cb = np.asarray(colbase, np.int64)
    col = cb[tt, w_of_edges] + jj
    flat[col * P + pp] = rows_of_edges.astype(np.int16)
    wrapped = np.tile(flat.reshape(-1, 16).T, (8, 1))
    return flat, wrapped


def _rank_within(a, b):
    """Stable rank of each element within its (a, b) group (a, b int arrays,
    pre-sorted arbitrary order)."""
    o = np.lexsort((b, a))
    sa, sb = a[o], b[o]
    change = np.ones(len(o), bool)
    change[1:] = (sa[1:] != sa[:-1]) | (sb[1:] != sb[:-1])
    starts = np.nonzero(change)[0]
    gid = np.cumsum(change) - 1
    rk = np.arange(len(o)) - starts[gid]
    out = np.empty(len(o), np.int64)
    out[o] = rk
    return out


def _preprocess(x, edge_index, batch, num_graphs, W0, b0, W1, b1, W2, b2):
    x = np.asarray(x, np.float32)
    N, IN = x.shape
    HID = W0.shape[1]
    G = int(num_graphs)
    SH = N // NCORES
    src = np.asarray(edge_index[0], np.int64)
    dst = np.asarray(edge_index[1], np.int64)
    batch = np.asarray(batch, np.int64)

    degi = np.bincount(dst, minlength=N) + 1          # + self-loop
    dis = (1.0 / np.sqrt(degi.astype(np.float64))).astype(np.float32)
    invd = np.sqrt(degi.astype(np.float64)).astype(np.float32)

    order = np.argsort(dst, kind="stable")
    s_sorted = src[order]
    indptr = np.searchsorted(dst, np.arange(N + 1), sorter=order)

    xhat = x * dis[:, None]

    # dense pooling matrix Mhat = diag(1/cnt) @ S_pool @ A_norm  [G, N]
    cnt = np.bincount(batch, minlength=G).astype(np.float64)
    cntc = np.maximum(cnt, 1.0)
    bd = batch[dst]
    w_ = dis[dst].astype(np.float64) * dis[src] / cntc[bd]
    M = np.bincount(bd * N + src, weights=w_, minlength=G * N)
    w2_ = dis.astype(np.float64) ** 2 / cntc[batch]
    M += np.bincount(batch * N + np.arange(N), weights=w2_, minlength=G * N)
    Mhat = M.reshape(G, N).astype(np.float32)

    # ---------------- per-core layer-0 structure (with pair matching) ----
    core_l0 = []
    pats0 = []
    for c in range(NCORES):
        own = np.arange(c * SH, (c + 1) * SH)
        esrc = s_sorted[indptr[c * SH]:indptr[(c + 1) * SH]]
        halo = np.unique(np.concatenate([esrc, own]))
        deg = indptr[halo + 1] - indptr[halo]          # ext in-deg
        E0 = int(deg.sum())
        rep = np.repeat(np.arange(len(halo)), deg)
        ei = np.repeat(indptr[halo], deg) + (
            np.arange(E0) - np.repeat(np.cumsum(deg) - deg, deg))
        l0src = s_sorted[ei]
        uniq, l0inv = np.unique(l0src, return_inverse=True)
        U = len(uniq)

        # Greedy pair packing: a 512-byte descriptor covers two of a dst's
        # sources stored adjacently; pair rows may share a source (rows are
        # duplicated as needed), so coverage is per-dst set packing.
        o = np.lexsort((l0inv, rep))
        rs, us = rep[o], l0inv[o]
        dgs = np.bincount(rs, minlength=len(halo))
        offs = np.concatenate([[0], np.cumsum(dgs)])
        occ_d, occ_a, occ_b = [], [], []
        for k1 in range(int(dgs.max())):
            for k2 in range(k1 + 1, int(dgs.max())):
                sel = dgs > k2
                i0 = offs[:-1][sel]
                a, b = us[i0 + k1], us[i0 + k2]
                ok = a != b
                occ_d.append(np.nonzero(sel)[0][ok])
                occ_a.append(np.minimum(a[ok], b[ok]))
                occ_b.append(np.maximum(a[ok], b[ok]))
        occ_d = np.concatenate(occ_d)
        occ_a = np.concatenate(occ_a)
        occ_b = np.concatenate(occ_b)
        okey = occ_a * U + occ_b
        uk, kinv, ucnt = np.unique(okey, return_inverse=True,
                                   return_counts=True)
        order_occ = np.lexsort((kinv, -ucnt[kinv]))

        dk = rep * U + l0inv
        udk, cnt_du = np.unique(dk, return_counts=True)
        avail = cnt_du.copy()
        ia_occ = np.searchsorted(udk, occ_d * U + occ_a)
        ib_occ = np.searchsorted(udk, occ_d * U + occ_b)

        rowid_of_key = {}
        prow_a, prow_b = [], []
        use_d, use_row, use_k = [], [], []
        for i in order_occ:
            ka = avail[ia_occ[i]]
            kb = avail[ib_occ[i]]
            k = ka if ka < kb else kb
            if k <= 0:
                continue
            rk = okey[i]
            rid = rowid_of_key.get(rk)
            if rid is None:
                rid = len(prow_a)
                rowid_of_key[rk] = rid
                prow_a.append(occ_a[i])
                prow_b.append(occ_b[i])
            avail[ia_occ[i]] = ka - k
            avail[ib_occ[i]] = kb - k
            use_d.append(occ_d[i])
            use_row.append(rid)
            use_k.append(k)
        NPAIR = len(prow_a)
        prow_a = np.array(prow_a, np.int64)
        prow_b = np.array(prow_b, np.int64)
        use_d = np.array(use_d, np.int64)
        use_row = np.array(use_row, np.int64)
        use_k = np.array(use_k, np.int64)

        # single uses = leftover availability per (dst, src)
        dd, uu = divmod(udk, U)
        sing_du = avail
        single_src_tot = np.bincount(uu, weights=sing_du.astype(np.float64),
                                     minlength=U).astype(np.int64)
        srow_ids = np.nonzero(single_src_tot > 0)[0]
        smap = np.full(U, -1, np.int64)
        smap[srow_ids] = np.arange(len(srow_ids))
        nrows = NPAIR + len(srow_ids)
        pair_pop = np.bincount(use_row, weights=use_k.astype(np.float64),
                               minlength=NPAIR).astype(np.int64)
        pop = np.concatenate([pair_pop, single_src_tot[srow_ids]])
        rank = np.empty(nrows, np.int64)
        rank[np.argsort(-pop, kind="stable")] = np.arange(nrows)
        row_w = rank // WIN0
        row_loc = rank % WIN0 + 1                      # +1: row 0 is zeros
        NW0c = int(row_w.max()) + 1 if nrows else 1

        # flat descriptor list: (halo idx, window, loc, is_pair)
        vv = sing_du > 0
        d_i = np.concatenate([np.repeat(use_d, use_k),
                              np.repeat(dd[vv], sing_du[vv])])
        d_row = np.concatenate([np.repeat(rank[:NPAIR][use_row], use_k),
                                np.repeat(rank[NPAIR + smap[uu[vv]]],
                                          sing_du[vv])])
        d_pair = np.concatenate([np.ones(int(use_k.sum()), np.int64),
                                 np.zeros(int(sing_du[vv].sum()), np.int64)])
        d_w = d_row // WIN0
        d_loc = d_row % WIN0 + 1

        pat = np.zeros((len(halo), 3), np.int64)
        np.add.at(pat, (d_i, d_w), 1)
        ppat = np.zeros((len(halo), 3), np.int64)
        np.add.at(ppat, (d_i[d_pair > 0], d_w[d_pair > 0]), 1)

        core_l0.append(dict(own=own, halo=halo, uniq=uniq, NW0c=NW0c,
                            d_i=d_i, d_w=d_w, d_loc=d_loc, d_pair=d_pair,
                            ppat=ppat, prow_a=prow_a, prow_b=prow_b,
                            srow_ids=srow_ids, row_w=row_w, row_loc=row_loc,
                            NPAIR=NPAIR))
        pats0.append(pat)

    NW0 = max(d["NW0c"] for d in core_l0)
    slotpat0, upat0, starts0, T0 = _budget_slots(pats0, 512)
    assert T0 * P <= 2 * WIN1, "layer-1 source exceeds two windows"
    D0 = slotpat0.reshape(T0, P, 3).max(axis=1)
    D0t = tuple(tuple(int(v) for v in row) for row in D0)
    _, calls0, colbase0, C0 = _schedule(D0t, NW0, GCOLS0, CCAP0)

    key0 = {tuple(p): i for i, p in enumerate(upat0)}

    # ---------------- per-core slot assignment + idx0 ----------------
    cores = []
    pats1 = []
    l1_parts = []
    P1_cores = []
    for c in range(NCORES):
        d = core_l0[c]
        halo, pat = d["halo"], pats0[c]
        cls = np.array([key0[tuple(p)] for p in pat], np.int64)
        # within a class, cluster similar pair-count vectors so the shared
        # plane-1 profile stays tight across cores
        pp = d["ppat"]
        o = np.lexsort((pp[:, 2], pp[:, 1], pp[:, 0], cls))
        pos0cls = np.searchsorted(cls[o], np.arange(len(upat0)))
        rank = np.empty(len(halo), np.int64)
        rank[o] = np.arange(len(halo)) - pos0cls[cls[o]]
        slot_of_halo = starts0[cls] + rank             # halo idx -> slot
        slot_node = np.full(T0 * P, -1, np.int64)
        slot_node[slot_of_halo] = halo

        # idx0 from the descriptor list; pair descs rank first per (slot, w)
        e_slot = slot_of_halo[d["d_i"]]
        e_w, e_loc, e_pair = d["d_w"], d["d_loc"], d["d_pair"]
        oo = np.lexsort((1 - e_pair, e_w, e_slot))
        so_s, so_w = e_slot[oo], e_w[oo]
        change = np.ones(len(oo), bool)
        change[1:] = (so_s[1:] != so_s[:-1]) | (so_w[1:] != so_w[:-1])
        startsr = np.nonzero(change)[0]
        gid = np.cumsum(change) - 1
        jj = np.empty(len(oo), np.int64)
        jj[oo] = np.arange(len(oo)) - startsr[gid]
        flat0, idx0 = _fill_idx(e_slot, e_loc, e_w, jj, colbase0, D0t,
                                C0, NW0)

        # per-tile pair-column profile (this core)
        pc_slot = np.zeros((T0 * P, 3), np.int64)
        pc_slot[slot_of_halo] = d["ppat"]
        P1_cores.append(pc_slot.reshape(T0, P, 3).max(axis=1))

        # xh pair table: row r = [xhat[a], xhat[b]] or [xhat[s], 0]
        uniq = d["uniq"]
        xh = np.zeros((NW0 * WSTR, 2 * IN), np.float32)
        rw, rl = d["row_w"], d["row_loc"]
        npair = d["NPAIR"]
        if npair:
            prow = rw[:npair] * WSTR + rl[:npair]
            xh[prow, :IN] = xhat[uniq[d["prow_a"]]]
            xh[prow, IN:] = xhat[uniq[d["prow_b"]]]
        if len(d["srow_ids"]):
            srow = rw[npair:] * WSTR + rl[npair:]
            xh[srow, :IN] = xhat[uniq[d["srow_ids"]]]
        xh = xh.astype(BF16)

        # xselfT: [IN, T0*P] column s = xhat[node(s)]
        xselfT = np.zeros((IN, T0 * P), np.float32)
        v = slot_node >= 0
        xselfT[:, v] = xhat[slot_node[v]].T
        xselfT = xselfT.astype(FP8)

        # per-slot scales
        dis0 = np.zeros(T0 * P, np.float32)
        dis0[v] = (dis * dis)[slot_node[v]]
        inv0 = np.zeros(T0 * P, np.float32)
        inv0[v] = invd[slot_node[v]]

        # ---------------- layer-1 structure ----------------
        own = d["own"]
        pos_of_node = np.full(N, -1, np.int64)
        pos_of_node[slot_node[v]] = np.nonzero(v)[0]
        degc = indptr[own + 1] - indptr[own]
        E1 = int(degc.sum())
        rep1 = np.repeat(np.arange(SH), degc)
        ei1 = np.repeat(indptr[own], degc) + (
            np.arange(E1) - np.repeat(np.cumsum(degc) - degc, degc))
        l1src = s_sorted[ei1]
        # append self edges
        rep1 = np.concatenate([rep1, np.arange(SH)])
        l1src = np.concatenate([l1src, own])
        spos = pos_of_node[l1src]
        assert (spos >= 0).all()
        w1 = spos // WIN1
        loc1 = _h1row(spos) - w1 * WSTR
        pat1 = np.zeros((SH, 2), np.int64)
        np.add.at(pat1, (rep1, w1), 1)
        pats1.append(pat1)
        l1_parts.append(dict(rep1=rep1, w1=w1, loc1=loc1, pat1=pat1))

        cores.append(dict(slot_node=slot_node, idx0=idx0, flat0=flat0,
                          xh=xh, xselfT=xselfT, dis0=dis0, inv0=inv0))

    P1 = np.maximum.reduce(P1_cores)
    P1 = np.minimum(P1, D0)                    # plane-1 loop bound <= D
    P1t = tuple(tuple(int(v) for v in row) for row in P1)

    NW1 = -(-(T0 * P) // WIN1)
    # layer-1 slots: per-core lexsort by pattern; shared profile = max
    T1 = -(-(-(-SH // P)) // 4) * 4
    profs1 = []
    orders1 = []
    for c in range(NCORES):
        pat1 = l1_parts[c]["pat1"]
        o = np.lexsort(pat1.T[::-1])
        orders1.append(o)
        padded = np.zeros((T1 * P, 2), np.int64)
        padded[T1 * P - SH:] = pat1[o]
        profs1.append(padded.reshape(T1, P, 2).max(axis=1))
    D1 = np.maximum.reduce(profs1)
    D1t = tuple(tuple(int(v) for v in row) for row in D1)
    _, calls1, colbase1, C1 = _schedule(D1t, NW1)

    for c in range(NCORES):
        d = l1_parts[c]
        own = core_l0[c]["own"]
        o = orders1[c]
        slot1_of_own = np.empty(SH, np.int64)
        slot1_of_own[o] = T1 * P - SH + np.arange(SH)
        slot1_node = np.full(T1 * P, -1, np.int64)
        slot1_node[slot1_of_own] = own

        e_slot = slot1_of_own[d["rep1"]]
        jj = _rank_within(e_slot, d["w1"])
        flat1, idx1 = _fill_idx(e_slot, d["loc1"], d["w1"], jj, colbase1,
                                D1t, C1, NW1)

        v1 = slot1_node >= 0
        dis1 = np.zeros(T1 * P, np.float32)
        dis1[v1] = dis[slot1_node[v1]]
        inv1 = np.zeros(T1 * P, np.float32)
        inv1[v1] = invd[slot1_node[v1]]

        # pooling matrix, partition-tiled: [P, T1*G]
        mctT = np.zeros((P, T1 * G), np.float32)
        sn = slot1_node.reshape(T1, P)
        for t in range(T1):
            vt = sn[t] >= 0
            mctT[vt, t * G:(t + 1) * G] = Mhat[:, sn[t][vt]].T
        # fp8 would quantize pooling weights ~4%; scale rows to use the
        # format's range better is unnecessary -- keep bf16
        cd = cores[c]
        cd.update(idx1=idx1, flat1=flat1,
                  dis0=cd["dis0"].reshape(T0, P).T.copy(),
                  dis1=dis1.reshape(T1, P).T.copy(),
                  inv0=cd["inv0"].reshape(1, T0 * P).astype(BF16),
                  inv1=inv1.reshape(1, T1 * P).astype(BF16),
                  mctT=mctT.astype(BF16), slot1_node=slot1_node)

    shared = dict(
        w0=np.ascontiguousarray(W0, np.float32).astype(BF16),
        w1=np.ascontiguousarray(W1, np.float32).reshape(2, P, HID).astype(BF16),
        b0r=np.ascontiguousarray(b0, np.float32).reshape(1, HID).astype(BF16),
        b1r=np.ascontiguousarray(b1, np.float32).reshape(1, HID).astype(BF16),
        ident=np.eye(P, dtype=np.float32).astype(BF16),
    )
    zero_bias = bool(np.all(np.asarray(b0) == 0) and np.all(np.asarray(b1) == 0))
    meta = dict(N=N, IN=IN, HID=HID, G=G, SH=SH, T0=T0, T1=T1,
                NW0=NW0, NW1=NW1, C0=C0, C1=C1, zero_bias=zero_bias,
                D0=D0t, D1=D1t, P1=P1t)
    fin = dict(W2=np.asarray(W2, np.float32), b2=np.asarray(b2, np.float32))
    return meta, shared, cores, fin


# ---------------------------------------------------------------------------
# Pure-numpy emulation of the device program (validation / debugging)
# ---------------------------------------------------------------------------

def _emulate(meta, shared, cores, fin):
    T0, T1, HID, IN, G = (meta[k] for k in ("T0", "T1", "HID", "IN", "G"))
    NW0, NW1 = meta["NW0"], meta["NW1"]
    D0, D1, P1 = meta["D0"], meta["D1"], meta["P1"]
    _, calls0, colbase0, C0 = _schedule(D0, NW0, GCOLS0, CCAP0)
    _, calls1, colbase1, C1 = _schedule(D1, NW1)
    w0 = shared["w0"].astype(np.float32)
    w1 = shared["w1"].astype(np.float32).reshape(2 * P, HID)
    b0 = shared["b0r"].astype(np.float32)[0]
    b1 = shared["b1r"].astype(np.float32)[0]

    def col_windows(calls, C):
        cw = np.zeros(C, np.int64)
        for w, n, off in calls:
            cw[off:off + n] = w
        return cw

    cw0 = col_windows(calls0, C0)
    cw1 = col_windows(calls1, C1)

    Y = np.zeros((G, HID), np.float32)
    for cd in cores:
        xh = cd["xh"].astype(np.float32)
        rows = cw0.repeat(P) * WSTR + cd["flat0"]
        gat = xh[rows].reshape(C0, P, 2 * IN)
        u0 = cd["xselfT"].astype(np.float32).T.copy()   # [T0*P, IN]
        for t in range(T0):
            for w in range(NW0):
                cb = colbase0[t][w]
                for j in range(D0[t][w]):
                    u0[t * P:(t + 1) * P] += gat[cb + j][:, :IN]
                    if j < P1[t][w]:
                        u0[t * P:(t + 1) * P] += gat[cb + j][:, IN:]
        u0 = u0.astype(BF16).astype(np.float32)
        inv0 = cd["inv0"].astype(np.float32)[0]
        pre = u0 @ w0 + inv0[:, None] * b0[None, :]
        d0 = cd["dis0"].T.reshape(-1)
        h1 = np.maximum(d0[:, None] * pre, 0).astype(FP8).astype(np.float32)
        # place h1 into windowed layout (grouped rows)
        h1w = np.zeros((NW1 * WSTR, HID), np.float32)
        h1w[_h1row(np.arange(T0 * P))] = h1

        rows1 = cw1.repeat(P) * WSTR + cd["flat1"]
        gat1 = h1w[rows1].reshape(C1, P, HID)
        u1 = np.zeros((T1 * P, HID), np.float32)
        for t in range(T1):
            for w in range(NW1):
                cb = colbase1[t][w]
                for j in range(D1[t][w]):
                    u1[t * P:(t + 1) * P] += gat1[cb + j]
        u1 = u1.astype(BF16).astype(np.float32)
        inv1 = cd["inv1"].astype(np.float32)[0]
        pre1 = u1 @ w1 + inv1[:, None] * b1[None, :]
        d1 = cd["dis1"].T.reshape(-1)
        h2 = np.maximum(d1[:, None] * pre1, 0).astype(BF16).astype(np.float32)
        mctT = cd["mctT"].astype(np.float32)
        for t in range(T1):
            Y += mctT[:, t * G:(t + 1) * G].T @ h2[t * P:(t + 1) * P]
    return Y @ fin["W2"] + fin["b2"]


# ---------------------------------------------------------------------------
# Bass device program
# ---------------------------------------------------------------------------

def _build(meta):
    import concourse.bass as bass
    import concourse.mybir as mybir
    import concourse.tile as tile
    from concourse import bacc, library_config
    from concourse.tile_rust import add_dep_helper

    F32, I16 = mybir.dt.float32, mybir.dt.int16
    BF = mybir.dt.bfloat16
    F8 = mybir.dt.float8e4
    RELU = mybir.ActivationFunctionType.Relu

    IN, HID, G = meta["IN"], meta["HID"], meta["G"]
    T0, T1 = meta["T0"], meta["T1"]
    NW0, NW1 = meta["NW0"], meta["NW1"]
    C0, C1 = meta["C0"], meta["C1"]
    D0, D1, P1 = meta["D0"], meta["D1"], meta["P1"]
    ZB = meta["zero_bias"]
    groups0, calls0, colbase0, _ = _schedule(D0, NW0, GCOLS0, CCAP0)
    groups1, calls1, colbase1, _ = _schedule(D1, NW1)
    c2c0 = _call_of_col(calls0)
    c2c1 = _call_of_col(calls1)

    def _max_calls_per_group(groups, colbase, D, NW, c2c):
        mx = 1
        for (t0, t1) in groups:
            ids = set()
            for tt in range(t0, t1):
                for w in range(NW):
                    for j in range(D[tt][w]):
                        ids.add(c2c[colbase[tt][w] + j][0])
            mx = max(mx, len(ids))
        return mx

    gbufs0 = _max_calls_per_group(groups0, colbase0, D0, NW0, c2c0) + 4
    gbufs1 = _max_calls_per_group(groups1, colbase1, D1, NW1, c2c1) + 4

    nc = bacc.Bacc("TRN2", target_bir_lowering=False, debug=False,
                   num_devices=NCORES)

    t_xh = nc.dram_tensor("xh", [NW0 * WSTR, 2 * IN], BF,
                          kind="ExternalInput")
    t_xst = nc.dram_tensor("xselfT", [IN, T0 * P], F8, kind="ExternalInput")
    t_idx0 = nc.dram_tensor("idx0", [P, C0 * 8], I16, kind="ExternalInput")
    t_idx1 = nc.dram_tensor("idx1", [P, C1 * 8], I16, kind="ExternalInput")
    t_dis0 = nc.dram_tensor("dis0", [P, T0], F32, kind="ExternalInput")
    t_dis1 = nc.dram_tensor("dis1", [P, T1], F32, kind="ExternalInput")
    t_inv0 = nc.dram_tensor("inv0", [1, T0 * P], BF, kind="ExternalInput")
    t_inv1 = nc.dram_tensor("inv1", [1, T1 * P], BF, kind="ExternalInput")
    t_w0 = nc.dram_tensor("w0", [IN, HID], BF, kind="ExternalInput")
    t_w1 = nc.dram_tensor("w1", [2, P, HID], BF, kind="ExternalInput")
    t_b0 = nc.dram_tensor("b0r", [1, HID], BF, kind="ExternalInput")
    t_b1 = nc.dram_tensor("b1r", [1, HID], BF, kind="ExternalInput")
    t_mct = nc.dram_tensor("mctT", [P, T1 * G], BF, kind="ExternalInput")
    t_id = nc.dram_tensor("ident", [P, P], BF, kind="ExternalInput")
    t_out = nc.dram_tensor("outp", [G, HID], F32, kind="ExternalOutput")

    with tile.TileContext(nc) as tc:
        with (
            tc.tile_pool(name="const", bufs=1) as cpool,
            tc.tile_pool(name="ut", bufs=4) as upool,
            tc.tile_pool(name="stage", bufs=4) as spool,
            tc.tile_pool(name="h2res", bufs=10) as h2pool,
            tc.tile_pool(name="aggps", bufs=3, space="PSUM") as apool,
            tc.tile_pool(name="agg1ps", bufs=2, space="PSUM") as a1pool,
            tc.tile_pool(name="preps", bufs=2, space="PSUM") as ppool,
            tc.tile_pool(name="outps", bufs=1, space="PSUM") as opool,
            tc.tile_pool(name="dram", bufs=1, space="DRAM") as dpool,
        ):
            lib = nc.gpsimd.load_library(library_config.mlp)

            def cload(t, shape, dt):
                s = cpool.tile(shape, dt, tag=t.name)
                nc.sync.dma_start(s[:], t[:])
                return s

            ident = cload(t_id, [P, P], BF)
            ident8 = cpool.tile([P, P], F8, tag="ident8")
            nc.vector.tensor_copy(ident8[:], ident[:])
            w0 = cload(t_w0, [IN, HID], BF)
            w1 = cpool.tile([P, 2, HID], BF, tag="w1")
            nc.sync.dma_start(w1[:], t_w1[:].rearrange("j p h -> p j h"))
            if not ZB:
                b0r = cload(t_b0, [1, HID], BF)
                b1r = cload(t_b1, [1, HID], BF)
            dis0 = cload(t_dis0, [P, T0], F32)
            dis1 = cload(t_dis1, [P, T1], F32)
            mt = cload(t_mct, [P, T1 * G], BF)

            h1h = dpool.tile([NW1 * WSTR, HID], F8)

            h1h_writes = []
            zt = spool.tile([P, HID], F8, tag="zrow")
            nc.vector.memset(zt[:], 0.0)
            for w in range(NW1):
                h1h_writes.append(nc.sync.dma_start(
                    h1h[w * WSTR:w * WSTR + 1, :], zt[0:1, :]))

            def slot_row(s):
                return (s // WIN1) * WSTR + 1 + (s % WIN1)

            def do_gathers(t_src_ap, idx_sb, calls, grp_calls, buf_pool,
                           ccap_elem, elem, deps, dt=BF):
                out = {}
                for ci in grp_calls:
                    w, ncols, off = calls[ci]
                    gt = buf_pool.tile([P, ccap_elem], dt, tag="g")
                    gi = nc.gpsimd.dma_gather(
                        gt[:, :ncols * elem].rearrange(
                            "p (j d) -> p j d", j=ncols),
                        t_src_ap(w),
                        idx_sb[:, off * 8:(off + ncols) * 8],
                        ncols * P, ncols * P, elem, single_packet=False)
                    add_dep_helper(gi.ins, lib.ins, True, "gather after lib")
                    for d in deps:
                        add_dep_helper(gi.ins, d.ins, True, "gather after src")
                    out[ci] = gt
                return out

            def grp_call_ids(t0, t1, colbase, D, NW, c2c):
                ids = set()
                for tt in range(t0, t1):
                    for w in range(NW):
                        for j in range(D[tt][w]):
                            ids.add(c2c[colbase[tt][w] + j][0])
                return sorted(ids)

            # ---------------- Layer 0 ----------------
            with tc.tile_pool(name="idx0p", bufs=1) as ipool0, \
                 tc.tile_pool(name="g0", bufs=gbufs0) as gpool0, \
                 tc.tile_pool(name="xs0", bufs=3) as xspool:
                idx0 = ipool0.tile([P, C0 * 8], I16, tag="idx0")
                nc.sync.dma_start(idx0[:], t_idx0[:])

                bank = None
                uT4 = None
                stage_t = None
                for (t0g, t1g) in groups0:
                    ids = grp_call_ids(t0g, t1g, colbase0, D0, NW0, c2c0)
                    bufs = do_gathers(
                        lambda w: t_xh[w * WSTR:(w + 1) * WSTR, :],
                        idx0, calls0, ids, gpool0, CCAP0 * 2 * IN, 2 * IN,
                        [])
                    # dense self-loop chunk for this group's tiles
                    xst = xspool.tile([P, (t1g - t0g) * P], F8, tag="xs")
                    nc.sync.dma_start(xst[:],
                                      t_xst[:, t0g * P:t1g * P])
                    for t in range(t0g, t1g):
                        q = t % 4
                        if q == 0:
                            bank = apool.tile([P, 4 * P], mybir.dt.float32,
                                              tag="aggps", space="PSUM")
                        ups = bank[:, q * P:(q + 1) * P]
                        nd = sum(D0[t]) + sum(min(P1[t][w], D0[t][w])
                                              for w in range(NW0))
                        # self contribution: ups = I @ xselfT_t
                        nc.tensor.matmul(
                            ups, lhsT=ident8[:],
                            rhs=xst[:, (t - t0g) * P:(t - t0g + 1) * P],
                            start=True, stop=(nd == 0))
                        k = 0
                        for w in range(NW0):
                            for j in range(D0[t][w]):
                                ci, lc = c2c0[colbase0[t][w] + j]
                                gt = bufs[ci]
                                nplanes = 2 if j < P1[t][w] else 1
                                for pl in range(nplanes):
                                    nc.tensor.matmul(
                                        ups,
                                        lhsT=gt[:, lc * 2 * IN + pl * IN:
                                                lc * 2 * IN + (pl + 1) * IN],
                                        rhs=ident[:], start=False,
                                        stop=(k == nd - 1))
                                    k += 1
                        if q == 3:
                            uT4 = upool.tile([P, 4 * P], BF, tag="ut")
                            nc.vector.tensor_copy(uT4[:], bank[:])
                        # defer pre/act until uT4 is ready
                        if q == 3:
                            for tq in range(t - 3, t + 1):
                                qq = tq % 4
                                pre = ppool.tile([P, HID], mybir.dt.float32,
                                                 tag="preps", space="PSUM")
                                if ZB:
                                    nc.tensor.matmul(
                                        pre[:],
                                        lhsT=uT4[:, qq * P:(qq + 1) * P],
                                        rhs=w0[:], start=True, stop=True)
                                else:
                                    sd = upool.tile([1, P], BF, tag="seed")
                                    nc.sync.dma_start(
                                        sd[:], t_inv0[:, tq * P:(tq + 1) * P])
                                    nc.tensor.matmul(pre[:], lhsT=sd[:],
                                                     rhs=b0r[:],
                                                     start=True, stop=False)
                                    nc.tensor.matmul(
                                        pre[:],
                                        lhsT=uT4[:, qq * P:(qq + 1) * P],
                                        rhs=w0[:], start=False, stop=True)
                                if qq == 0:
                                    stage_t = spool.tile([P, 4 * HID], F8,
                                                         tag="h1stage")
                                nc.scalar.activation(
                                    stage_t[:, qq * HID:(qq + 1) * HID],
                                    pre[:], RELU, bias=0.0,
                                    scale=dis0[:, tq:tq + 1])
                                if qq == 3:
                                    r0 = slot_row((tq - 3) * P)
                                    h1h_writes.append(nc.sync.dma_start(
                                        h1h[r0:r0 + 4 * P, :].rearrange(
                                            "(p j) h -> p j h", p=P),
                                        stage_t[:].rearrange(
                                            "p (j h) -> p j h", j=4)))

            # ---------------- Layer 1 ----------------
            with tc.tile_pool(name="idx1p", bufs=1) as ipool1, \
                 tc.tile_pool(name="g1", bufs=gbufs1) as gpool1:
                idx1 = ipool1.tile([P, C1 * 8], I16, tag="idx1")
                nc.sync.dma_start(idx1[:], t_idx1[:])

                bank = None
                h2_tiles = {}
                pooled = 0
                ops = opool.tile([G, HID], mybir.dt.float32,
                                 tag="outps", space="PSUM")

                def pool_flush(upto):
                    nonlocal pooled
                    while pooled < upto:
                        tp = pooled
                        ht = h2_tiles.pop(tp)
                        nc.tensor.matmul(
                            ops[:], lhsT=mt[:, tp * G:(tp + 1) * G],
                            rhs=ht[:, (tp % 2) * HID:(tp % 2 + 1) * HID],
                            start=(tp == 0), stop=(tp == T1 - 1))
                        pooled += 1

                for (t0g, t1g) in groups1:
                    ids = grp_call_ids(t0g, t1g, colbase1, D1, NW1, c2c1)
                    bufs = do_gathers(
                        lambda w: h1h[w * WSTR:(w + 1) * WSTR, :],
                        idx1, calls1, ids, gpool1, CCAP * HID, HID,
                        h1h_writes, dt=F8)
                    for t in range(t0g, t1g):
                        q = t % 2
                        if q == 0:
                            bank = a1pool.tile([P, 4 * P], mybir.dt.float32,
                                               tag="agg1ps", space="PSUM")
                        u0ps = bank[:, q * HID:q * HID + P]
                        u1ps = bank[:, q * HID + P:(q + 1) * HID]
                        nd = sum(D1[t])
                        if nd == 0:
                            nc.tensor.matmul(u0ps, lhsT=ident[:],
                                             rhs=ident[:], start=True,
                                             stop=True)
                            nc.tensor.matmul(u1ps, lhsT=ident[:],
                                             rhs=ident[:], start=True,
                                             stop=True)
                        for half, ups in ((0, u0ps), (1, u1ps)):
                            k = 0
                            for w in range(NW1):
                                for j in range(D1[t][w]):
                                    ci, lc = c2c1[colbase1[t][w] + j]
                                    gt = bufs[ci]
                                    nc.tensor.matmul(
                                        ups,
                                        lhsT=gt[:, lc * HID + half * P:
                                                lc * HID + (half + 1) * P],
                                        rhs=ident8[:], start=(k == 0),
                                        stop=(k == nd - 1))
                                    k += 1
                        if q == 1:
                            uT4 = upool.tile([P, 4 * P], BF, tag="ut")
                            nc.vector.tensor_copy(uT4[:], bank[:])
                            for tq in (t - 1, t):
                                qq = tq % 2
                                pre = ppool.tile([P, HID], mybir.dt.float32,
                                                 tag="preps", space="PSUM")
                                if not ZB:
                                    sd = upool.tile([1, P], BF, tag="seed")
                                    nc.sync.dma_start(
                                        sd[:], t_inv1[:, tq * P:(tq + 1) * P])
                                    nc.tensor.matmul(pre[:], lhsT=sd[:],
                                                     rhs=b1r[:],
                                                     start=True, stop=False)
                                nc.tensor.matmul(
                                    pre[:],
                                    lhsT=uT4[:, qq * HID:qq * HID + P],
                                    rhs=w1[:, 0, :], start=ZB, stop=False)
                                nc.tensor.matmul(
                                    pre[:],
                                    lhsT=uT4[:, qq * HID + P:(qq + 1) * HID],
                                    rhs=w1[:, 1, :], start=False, stop=True)
                                if tq % 2 == 0:
                                    h2_t = h2pool.tile([P, 2 * HID], BF,
                                                       tag="h2s")
                                    h2_tiles[tq] = h2_tiles[tq + 1] = h2_t
                                nc.scalar.activation(
                                    h2_tiles[tq][:, (tq % 2) * HID:
                                                 (tq % 2 + 1) * HID],
                                    pre[:], RELU, bias=0.0,
                                    scale=dis1[:, tq:tq + 1])
                                # pooling matmuls lag 16 tiles behind so
                                # they never wait on a fresh activation
                                pool_flush(max(0, tq - 15))

                pool_flush(T1)
                osb = spool.tile([G, HID], mybir.dt.float32, tag="osb")
                nc.vector.tensor_copy(osb[:], ops[:])
                nc.sync.dma_start(t_out[:], osb[:])

    nc.compile()
    return nc


# ---------------------------------------------------------------------------
# Entry point
# ---------------------------------------------------------------------------

_cache = {}


def _get_nc(meta):
    key = hashlib.sha1(repr(sorted(meta.items())).encode()).hexdigest()
    if key not in _cache:
        _cache[key] = _build(meta)
    return _cache[key]


def _in_maps(shared, cores):
    maps = []
    for cd in cores:
        m = dict(shared)
        m.update({k: cd[k] for k in
                  ("xh", "xselfT", "idx0", "idx1", "dis0", "dis1",
                   "inv0", "inv1", "mctT")})
        maps.append(m)
    return maps


def _run_device(meta, shared, cores):
    from concourse.bass_utils import run_bass_kernel_spmd
    nc = _get_nc(meta)
    res = run_bass_kernel_spmd(nc, _in_maps(shared, cores),
                               core_ids=list(range(NCORES)))
    return [r["outp"] for r in res.results]


def kernel(**inputs):
    meta, shared, cores, fin = _preprocess(**inputs)
    outs = _run_device(meta, shared, cores)
    Y = np.sum(np.stack(outs), axis=0, dtype=np.float32)
    out = Y @ fin["W2"] + fin["b2"]
    return out.astype(np.float32)


def profile_run(meta, shared, cores, trace_cores=None):
    """Profiled exec time in ns: NTFF trace when available, else the
    instruction-cost-model timeline simulation of the compiled program."""
    from concourse.bass_utils import run_bass_kernel_spmd
    nc = _get_nc(meta)
    try:
        res = run_bass_kernel_spmd(nc, _in_maps(shared, cores),
                                   core_ids=list(range(NCORES)), trace=True,
                                   trace_cores=trace_cores)
        if res.exec_time_ns is not None:
            print("profile:", res.instructions_and_trace[1]
                  if res.instructions_and_trace else None)
            return res.exec_time_ns
    except Exception as e:
        print(f"NTFF trace unavailable ({type(e).__name__}); "
              "using cost-model timeline")
    from concourse.timeline_sim import TimelineSim
    ts = TimelineSim(nc, trace=False)
    ts.simulate()
    return int(ts.time)


# revision 80
# speedup vs baseline: 1.0160x; 1.0160x over previous
"""GCN (3-layer GCNConv + global mean pool) on 8 Trainium2 NeuronCores.

Math: with S = adjacency+self-loops and D = diag(1/sqrt(deg)),
    conv(h) = relu(D S D h W + b)
and the diagonal scalings commute with the dense W, so each layer is an
UNWEIGHTED gather-sum of pre-scaled rows plus a dense matmul.  The final
conv + mean-pool collapse into a dense host-precomputed pooling matrix:
    out = (Mhat @ h2) @ W2 + b2,   Mhat = diag(1/cnt) S_pool A_norm.

Sharding: nodes dst-partitioned across 8 cores; layer 0 is recomputed on
each core's halo (src nodes of its incident edges) so cores never
communicate.  Optimizations over the plain halo design:
  * per-core xh holds only the deduplicated layer-0 edge sources (3
    int16-index windows, placed by local out-degree), so gather columns
    carry far less padding than a shared node-id-ordered table;
  * self-loop contributions stream in as a dense pre-transposed
    [feat, slot] table (bytes-bound DMA) and enter PSUM via one
    identity matmul per tile instead of per-slot gather descriptors;
  * slot schedules are budgeted per window-count pattern across cores so
    all 8 cores share one compiled program with an exact column profile;
  * PSUM aggregation packs 4 tiles per bank, copied out in one DVE op;
  * h2 never round-trips DRAM: layer-1 outputs stay in SBUF and feed the
    pooling matmuls directly, with the pooling matrix staged in a
    partition-tiled layout (large DMA descriptors).
"""

import hashlib

import numpy as np
import ml_dtypes

P = 128
NCORES = 8
WSTR = 32768      # physical window stride (rows); row w*WSTR is all-zero
WIN0 = 32767      # usable rows per layer-0 source window
WIN1 = 32256      # usable slots per layer-1 source window (512-aligned)
GCOLS = 96        # gather column budget per tile-group (layer 1)
CCAP = 32         # max columns per dma_gather call (layer 1)
GCOLS0 = 64       # layer-0 columns are 512B pair-reads: smaller groups
CCAP0 = 16

BF16 = ml_dtypes.bfloat16
FP8 = ml_dtypes.float8_e4m3


def _h1row(s):
    """Slot -> h1h row (grouped so that one partition's 4 rows of a write
    block are contiguous: 1KB fp8 descriptors).  WIN1 % 512 == 0 keeps
    512-slot blocks window-aligned."""
    w = s // WIN1
    loc = s - w * WIN1
    b = loc // 512
    p = s % P
    j = (s // P) % 4
    return w * WSTR + 1 + b * 512 + p * 4 + j


# ---------------------------------------------------------------------------
# Shared schedule derivation (host + builder + emulator all use this)
# ---------------------------------------------------------------------------

def _schedule(D, NW, gcols=GCOLS, ccap=CCAP):
    """D: [T][NW] column counts.  Returns (groups, calls, colbase, Ctot).

    groups: list of (t0, t1) tile ranges with total columns <= gcols.
    calls: list of (w, ncols, col_off) in global column order; a call's
        columns are consecutive.  Global column order: per group, per
        window (ascending), per tile (ascending), per j.
    colbase: [T][NW] global column offset of (t, w)'s first column.
    """
    T = len(D)
    groups = []
    t = 0
    while t < T:
        # taper: the last tiles get half-size groups so the compute tail
        # after the final gather stays short
        lim = gcols if t < T - 24 else max(gcols // 4, 8)
        tot = sum(D[t])
        t1 = t + 1
        while t1 < T and tot + sum(D[t1]) <= lim:
            tot += sum(D[t1])
            t1 += 1
        groups.append((t, t1))
        t = t1
    colbase = [[0] * NW for _ in range(T)]
    calls = []
    off = 0
    for (t0, t1) in groups:
        for w in range(NW):
            cols = 0
            for tt in range(t0, t1):
                colbase[tt][w] = off + cols
                cols += D[tt][w]
            c0 = 0
            while c0 < cols:
                n = min(ccap, cols - c0)
                calls.append((w, n, off + c0))
                c0 += n
            off += cols
    return groups, calls, colbase, off


def _call_of_col(calls):
    """Map global column -> (call_id, local_col)."""
    m = {}
    for ci, (w, n, off) in enumerate(calls):
        for j in range(n):
            m[off + j] = (ci, j)
    return m


# ---------------------------------------------------------------------------
# Host preprocessing
# ---------------------------------------------------------------------------

def _budget_slots(pats_per_core, round_slots):
    """Shared pattern budget: every distinct window-count vector gets
    max-over-cores slots at a FIXED (lexicographically ordered) position.
    Returns (slotpat [T*P, NW], class ranges dict pat->slice, T)."""
    cat = np.concatenate(pats_per_core)
    upat, inv = np.unique(cat, axis=0, return_inverse=True)
    npat = len(upat)
    counts = np.zeros((len(pats_per_core), npat), np.int64)
    off = 0
    for c, p in enumerate(pats_per_core):
        counts[c] = np.bincount(inv[off:off + len(p)], minlength=npat)
        off += len(p)
    budget = counts.max(axis=0)
    lex = np.lexsort(upat.T[::-1])
    tot = int(budget.sum())
    T = -(-tot // round_slots) * (round_slots // P)
    slotpat = np.zeros((T * P, upat.shape[1]), np.int64)
    starts = np.zeros(npat, np.int64)
    pos = T * P - tot          # pads first
    for pi in lex:
        slotpat[pos:pos + budget[pi]] = upat[pi]
        starts[pi] = pos
        pos += budget[pi]
    return slotpat, upat, starts, T


def _fill_idx(slots_of_edges, rows_of_edges, w_of_edges, jj, colbase, D,
              Ctot, NW):
    """Build the flat int16 gather index list [Ctot*128] (0 = window zero
    row), then wrap for dma_gather: [128, Ctot*8].

    slots_of_edges: slot id per edge; rows_of_edges: +1-based row within
    window; w_of_edges: window id; jj: stable per-(slot,window) rank.
    """
    flat = np.zeros(Ctot * P, np.int16)
    tt = slots_of_edges // P
    pp = slots_of_edges % P
    cb = np.asarray(colbase, np.int64)
    col = cb[tt, w_of_edges] + jj
    flat[col * P + pp] = rows_of_edges.astype(np.int16)
    wrapped = np.tile(flat.reshape(-1, 16).T, (8, 1))
    return flat, wrapped


def _rank_within(a, b):
    """Stable rank of each element within its (a, b) group (a, b int arrays,
    pre-sorted arbitrary order)."""
    o = np.lexsort((b, a))
    sa, sb = a[o], b[o]
    change = np.ones(len(o), bool)
    change[1:] = (sa[1:] != sa[:-1]) | (sb[1:] != sb[:-1])
    starts = np.nonzero(change)[0]
    gid = np.cumsum(change) - 1
    rk = np.arange(len(o)) - starts[gid]
    out = np.empty(len(o), np.int64)
    out[o] = rk
    return out


def _preprocess(x, edge_index, batch, num_graphs, W0, b0, W1, b1, W2, b2):
    x = np.asarray(x, np.float32)
    N, IN = x.shape
    HID = W0.shape[1]
    G = int(num_graphs)
    SH = N // NCORES
    src = np.asarray(edge_index[0], np.int64)
    dst = np.asarray(edge_index[1], np.int64)
    batch = np.asarray(batch, np.int64)

    degi = np.bincount(dst, minlength=N) + 1          # + self-loop
    dis = (1.0 / np.sqrt(degi.astype(np.float64))).astype(np.float32)
    invd = np.sqrt(degi.astype(np.float64)).astype(np.float32)

    order = np.argsort(dst, kind="stable")
    s_sorted = src[order]
    indptr = np.searchsorted(dst, np.arange(N + 1), sorter=order)

    xhat = x * dis[:, None]

    # dense pooling matrix Mhat = diag(1/cnt) @ S_pool @ A_norm  [G, N]
    cnt = np.bincount(batch, minlength=G).astype(np.float64)
    cntc = np.maximum(cnt, 1.0)
    bd = batch[dst]
    w_ = dis[dst].astype(np.float64) * dis[src] / cntc[bd]
    M = np.bincount(bd * N + src, weights=w_, minlength=G * N)
    w2_ = dis.astype(np.float64) ** 2 / cntc[batch]
    M += np.bincount(batch * N + np.arange(N), weights=w2_, minlength=G * N)
    Mhat = M.reshape(G, N).astype(np.float32)

    # ---------------- per-core layer-0 structure (with pair matching) ----
    core_l0 = []
    pats0 = []
    for c in range(NCORES):
        own = np.arange(c * SH, (c + 1) * SH)
        esrc = s_sorted[indptr[c * SH]:indptr[(c + 1) * SH]]
        halo = np.unique(np.concatenate([esrc, own]))
        deg = indptr[halo + 1] - indptr[halo]          # ext in-deg
        E0 = int(deg.sum())
        rep = np.repeat(np.arange(len(halo)), deg)
        ei = np.repeat(indptr[halo], deg) + (
            np.arange(E0) - np.repeat(np.cumsum(deg) - deg, deg))
        l0src = s_sorted[ei]
        uniq, l0inv = np.unique(l0src, return_inverse=True)
        U = len(uniq)

        # Greedy pair packing: a 512-byte descriptor covers two of a dst's
        # sources stored adjacently; pair rows may share a source (rows are
        # duplicated as needed), so coverage is per-dst set packing.
        o = np.lexsort((l0inv, rep))
        rs, us = rep[o], l0inv[o]
        dgs = np.bincount(rs, minlength=len(halo))
        offs = np.concatenate([[0], np.cumsum(dgs)])
        occ_d, occ_a, occ_b = [], [], []
        for k1 in range(int(dgs.max())):
            for k2 in range(k1 + 1, int(dgs.max())):
                sel = dgs > k2
                i0 = offs[:-1][sel]
                a, b = us[i0 + k1], us[i0 + k2]
                ok = a != b
                occ_d.append(np.nonzero(sel)[0][ok])
                occ_a.append(np.minimum(a[ok], b[ok]))
                occ_b.append(np.maximum(a[ok], b[ok]))
        occ_d = np.concatenate(occ_d)
        occ_a = np.concatenate(occ_a)
        occ_b = np.concatenate(occ_b)
        okey = occ_a * U + occ_b
        uk, kinv, ucnt = np.unique(okey, return_inverse=True,
                                   return_counts=True)
        order_occ = np.lexsort((kinv, -ucnt[kinv]))

        dk = rep * U + l0inv
        udk, cnt_du = np.unique(dk, return_counts=True)
        avail = cnt_du.copy()
        ia_occ = np.searchsorted(udk, occ_d * U + occ_a)
        ib_occ = np.searchsorted(udk, occ_d * U + occ_b)

        rowid_of_key = {}
        prow_a, prow_b = [], []
        use_d, use_row, use_k = [], [], []
        for i in order_occ:
            ka = avail[ia_occ[i]]
            kb = avail[ib_occ[i]]
            k = ka if ka < kb else kb
            if k <= 0:
                continue
            rk = okey[i]
            rid = rowid_of_key.get(rk)
            if rid is None:
                rid = len(prow_a)
                rowid_of_key[rk] = rid
                prow_a.append(occ_a[i])
                prow_b.append(occ_b[i])
            avail[ia_occ[i]] = ka - k
            avail[ib_occ[i]] = kb - k
            use_d.append(occ_d[i])
            use_row.append(rid)
            use_k.append(k)
        NPAIRROW = len(prow_a)
        prow_a = np.array(prow_a, np.int64)
        prow_b = np.array(prow_b, np.int64)
        use_d = np.array(use_d, np.int64)
        use_row = np.array(use_row, np.int64)
        use_k = np.array(use_k, np.int64)

        # tier 2: merge two pair-uses of one dst into a quad descriptor (a
        # 512B fp8 row holds 4 sources)
        pu_d = np.repeat(use_d, use_k)
        pu_row = np.repeat(use_row, use_k)
        o2 = np.lexsort((pu_row, pu_d))
        pu_d, pu_row = pu_d[o2], pu_row[o2]
        runc = np.ones(len(pu_d), bool)
        runc[1:] = pu_d[1:] != pu_d[:-1]
        rstart = np.nonzero(runc)[0]
        rix = np.arange(len(pu_d)) - rstart[np.cumsum(runc) - 1]
        is_even = (rix % 2 == 0)
        has_next = np.zeros(len(pu_d), bool)
        if len(pu_d) > 1:
            has_next[:-1] = is_even[:-1] & (pu_d[:-1] == pu_d[1:])
        qa_idx = np.nonzero(has_next)[0]
        q_d = pu_d[qa_idx]
        q_r1 = pu_row[qa_idx]
        q_r2 = pu_row[qa_idx + 1]
        qlo = np.minimum(q_r1, q_r2)
        qhi = np.maximum(q_r1, q_r2)
        qkey = qlo * NPAIRROW + qhi if NPAIRROW else qlo
        uqk, qinv = np.unique(qkey, return_inverse=True)
        NQUAD = len(uqk)
        quad_p1 = uqk // max(NPAIRROW, 1)
        quad_p2 = uqk % max(NPAIRROW, 1)
        # leftover pair-uses (not merged)
        merged = np.zeros(len(pu_d), bool)
        merged[qa_idx] = True
        merged[qa_idx + 1] = True
        lp_d = pu_d[~merged]
        lp_row = pu_row[~merged]
        # pair rows still referenced directly
        pair_used = np.zeros(NPAIRROW, bool)
        pair_used[lp_row] = True
        pmap = np.full(NPAIRROW, -1, np.int64)
        pmap[pair_used] = np.arange(int(pair_used.sum()))
        NPAIR = int(pair_used.sum())

        # single uses = leftover availability per (dst, src)
        dd, uu = divmod(udk, U)
        sing_du = avail
        single_src_tot = np.bincount(uu, weights=sing_du.astype(np.float64),
                                     minlength=U).astype(np.int64)
        srow_ids = np.nonzero(single_src_tot > 0)[0]
        smap = np.full(U, -1, np.int64)
        smap[srow_ids] = np.arange(len(srow_ids))
        nrows = NQUAD + NPAIR + len(srow_ids)
        quad_pop = np.bincount(qinv, minlength=NQUAD).astype(np.int64)
        pair_pop = np.bincount(pmap[lp_row], minlength=NPAIR).astype(np.int64)
        pop = np.concatenate([quad_pop, pair_pop, single_src_tot[srow_ids]])
        rank = np.empty(nrows, np.int64)
        rank[np.argsort(-pop, kind="stable")] = np.arange(nrows)
        row_w = rank // WIN0
        row_loc = rank % WIN0 + 1                      # +1: row 0 is zeros
        NW0c = int(row_w.max()) + 1 if nrows else 1

        # flat descriptor list: (halo idx, window, loc, nplanes)
        vv = sing_du > 0
        d_i = np.concatenate([q_d, lp_d, np.repeat(dd[vv], sing_du[vv])])
        d_row = np.concatenate([rank[qinv],
                                rank[NQUAD + pmap[lp_row]],
                                np.repeat(rank[NQUAD + NPAIR +
                                               smap[uu[vv]]],
                                          sing_du[vv])])
        d_np = np.concatenate([np.full(len(q_d), 4, np.int64),
                               np.full(len(lp_d), 2, np.int64),
                               np.ones(int(sing_du[vv].sum()), np.int64)])
        d_w = d_row // WIN0
        d_loc = d_row % WIN0 + 1

        pat = np.zeros((len(halo), 3), np.int64)
        np.add.at(pat, (d_i, d_w), 1)
        qpat = np.zeros((len(halo), 3), np.int64)
        np.add.at(qpat, (d_i[d_np == 4], d_w[d_np == 4]), 1)
        ppat = np.zeros((len(halo), 3), np.int64)
        np.add.at(ppat, (d_i[d_np == 2], d_w[d_np == 2]), 1)

        core_l0.append(dict(own=own, halo=halo, uniq=uniq, NW0c=NW0c,
                            d_i=d_i, d_w=d_w, d_loc=d_loc, d_np=d_np,
                            qpat=qpat, ppat=ppat,
                            prow_a=prow_a, prow_b=prow_b,
                            quad_p1=quad_p1, quad_p2=quad_p2,
                            pair_used=pair_used, pmap=pmap,
                            srow_ids=srow_ids, row_w=row_w, row_loc=row_loc,
                            NQUAD=NQUAD, NPAIR=NPAIR))
        pats0.append(pat)

    NW0 = max(d["NW0c"] for d in core_l0)
    slotpat0, upat0, starts0, T0 = _budget_slots(pats0, 512)
    assert T0 * P <= 2 * WIN1, "layer-1 source exceeds two windows"
    D0 = slotpat0.reshape(T0, P, 3).max(axis=1)
    D0t = tuple(tuple(int(v) for v in row) for row in D0)
    _, calls0, colbase0, C0 = _schedule(D0t, NW0, GCOLS0, CCAP0)

    key0 = {tuple(p): i for i, p in enumerate(upat0)}

    # ---------------- per-core slot assignment + idx0 ----------------
    cores = []
    pats1 = []
    l1_parts = []
    P1_cores = []
    Q1_cores = []
    for c in range(NCORES):
        d = core_l0[c]
        halo, pat = d["halo"], pats0[c]
        cls = np.array([key0[tuple(p)] for p in pat], np.int64)
        # within a class, cluster similar quad/pair-count vectors so the
        # shared plane profiles stay tight across cores
        qp, pp = d["qpat"], d["ppat"]
        o = np.lexsort((pp[:, 2], pp[:, 1], pp[:, 0],
                        qp[:, 2], qp[:, 1], qp[:, 0], cls))
        pos0cls = np.searchsorted(cls[o], np.arange(len(upat0)))
        rank = np.empty(len(halo), np.int64)
        rank[o] = np.arange(len(halo)) - pos0cls[cls[o]]
        slot_of_halo = starts0[cls] + rank             # halo idx -> slot
        slot_node = np.full(T0 * P, -1, np.int64)
        slot_node[slot_of_halo] = halo

        # idx0 from the descriptor list; higher-plane descs rank first
        e_slot = slot_of_halo[d["d_i"]]
        e_w, e_loc, e_np = d["d_w"], d["d_loc"], d["d_np"]
        oo = np.lexsort((-e_np, e_w, e_slot))
        so_s, so_w = e_slot[oo], e_w[oo]
        change = np.ones(len(oo), bool)
        change[1:] = (so_s[1:] != so_s[:-1]) | (so_w[1:] != so_w[:-1])
        startsr = np.nonzero(change)[0]
        gid = np.cumsum(change) - 1
        jj = np.empty(len(oo), np.int64)
        jj[oo] = np.arange(len(oo)) - startsr[gid]
        flat0, idx0 = _fill_idx(e_slot, e_loc, e_w, jj, colbase0, D0t,
                                C0, NW0)

        # per-tile quad/pair column profiles (this core)
        qc_slot = np.zeros((T0 * P, 3), np.int64)
        qc_slot[slot_of_halo] = d["qpat"]
        Q1_cores.append(qc_slot.reshape(T0, P, 3).max(axis=1))
        pc_slot = np.zeros((T0 * P, 3), np.int64)
        pc_slot[slot_of_halo] = d["qpat"] + d["ppat"]
        P1_cores.append(pc_slot.reshape(T0, P, 3).max(axis=1))

        # xh row table (fp8): quad rows hold 4 sources, pair rows 2,
        # single rows 1 (rest zero)
        uniq = d["uniq"]
        xh = np.zeros((NW0 * WSTR, 4 * IN), np.float32)
        rw, rl = d["row_w"], d["row_loc"]
        nq, npr = d["NQUAD"], d["NPAIR"]
        if nq:
            qrow = rw[:nq] * WSTR + rl[:nq]
            xh[qrow, :IN] = xhat[uniq[d["prow_a"][d["quad_p1"]]]]
            xh[qrow, IN:2 * IN] = xhat[uniq[d["prow_b"][d["quad_p1"]]]]
            xh[qrow, 2 * IN:3 * IN] = xhat[uniq[d["prow_a"][d["quad_p2"]]]]
            xh[qrow, 3 * IN:] = xhat[uniq[d["prow_b"][d["quad_p2"]]]]
        if npr:
            pids = np.nonzero(d["pair_used"])[0]
            prow = rw[nq:nq + npr] * WSTR + rl[nq:nq + npr]
            xh[prow, :IN] = xhat[uniq[d["prow_a"][pids]]]
            xh[prow, IN:2 * IN] = xhat[uniq[d["prow_b"][pids]]]
        if len(d["srow_ids"]):
            srow = rw[nq + npr:] * WSTR + rl[nq + npr:]
            xh[srow, :IN] = xhat[uniq[d["srow_ids"]]]
        xh = xh.astype(FP8)

        # xselfT: [IN, T0*P] column s = xhat[node(s)]
        xselfT = np.zeros((IN, T0 * P), np.float32)
        v = slot_node >= 0
        xselfT[:, v] = xhat[slot_node[v]].T
        xselfT = xselfT.astype(FP8)

        # per-slot scales
        dis0 = np.zeros(T0 * P, np.float32)
        dis0[v] = (dis * dis)[slot_node[v]]
        inv0 = np.zeros(T0 * P, np.float32)
        inv0[v] = invd[slot_node[v]]

        # ---------------- layer-1 structure ----------------
        own = d["own"]
        pos_of_node = np.full(N, -1, np.int64)
        pos_of_node[slot_node[v]] = np.nonzero(v)[0]
        degc = indptr[own + 1] - indptr[own]
        E1 = int(degc.sum())
        rep1 = np.repeat(np.arange(SH), degc)
        ei1 = np.repeat(indptr[own], degc) + (
            np.arange(E1) - np.repeat(np.cumsum(degc) - degc, degc))
        l1src = s_sorted[ei1]
        # append self edges
        rep1 = np.concatenate([rep1, np.arange(SH)])
        l1src = np.concatenate([l1src, own])
        spos = pos_of_node[l1src]
        assert (spos >= 0).all()
        w1 = spos // WIN1
        loc1 = _h1row(spos) - w1 * WSTR
        pat1 = np.zeros((SH, 2), np.int64)
        np.add.at(pat1, (rep1, w1), 1)
        pats1.append(pat1)
        l1_parts.append(dict(rep1=rep1, w1=w1, loc1=loc1, pat1=pat1))

        cores.append(dict(slot_node=slot_node, idx0=idx0, flat0=flat0,
                          xh=xh, xselfT=xselfT, dis0=dis0, inv0=inv0))

    P1 = np.maximum.reduce(P1_cores)
    P1 = np.minimum(P1, D0)                    # 2-plane loop bound <= D
    P1t = tuple(tuple(int(v) for v in row) for row in P1)
    Q1 = np.maximum.reduce(Q1_cores)
    Q1 = np.minimum(Q1, P1)                    # 4-plane loop bound <= P1
    Q1t = tuple(tuple(int(v) for v in row) for row in Q1)

    NW1 = -(-(T0 * P) // WIN1)
    # layer-1 slots: per-core lexsort by pattern; shared profile = max
    T1 = -(-(-(-SH // P)) // 4) * 4
    profs1 = []
    orders1 = []
    for c in range(NCORES):
        pat1 = l1_parts[c]["pat1"]
        o = np.lexsort(pat1.T[::-1])
        orders1.append(o)
        padded = np.zeros((T1 * P, 2), np.int64)
        padded[T1 * P - SH:] = pat1[o]
        profs1.append(padded.reshape(T1, P, 2).max(axis=1))
    D1 = np.maximum.reduce(profs1)
    D1t = tuple(tuple(int(v) for v in row) for row in D1)
    _, calls1, colbase1, C1 = _schedule(D1t, NW1)

    for c in range(NCORES):
        d = l1_parts[c]
        own = core_l0[c]["own"]
        o = orders1[c]
        slot1_of_own = np.empty(SH, np.int64)
        slot1_of_own[o] = T1 * P - SH + np.arange(SH)
        slot1_node = np.full(T1 * P, -1, np.int64)
        slot1_node[slot1_of_own] = own

        e_slot = slot1_of_own[d["rep1"]]
        jj = _rank_within(e_slot, d["w1"])
        flat1, idx1 = _fill_idx(e_slot, d["loc1"], d["w1"], jj, colbase1,
                                D1t, C1, NW1)

        v1 = slot1_node >= 0
        dis1 = np.zeros(T1 * P, np.float32)
        dis1[v1] = dis[slot1_node[v1]]
        inv1 = np.zeros(T1 * P, np.float32)
        inv1[v1] = invd[slot1_node[v1]]

        # pooling matrix, partition-tiled: [P, T1*G]
        mctT = np.zeros((P, T1 * G), np.float32)
        sn = slot1_node.reshape(T1, P)
        for t in range(T1):
            vt = sn[t] >= 0
            mctT[vt, t * G:(t + 1) * G] = Mhat[:, sn[t][vt]].T
        # fp8 would quantize pooling weights ~4%; scale rows to use the
        # format's range better is unnecessary -- keep bf16
        cd = cores[c]
        cd.update(idx1=idx1, flat1=flat1,
                  dis0=cd["dis0"].reshape(T0, P).T.copy(),
                  dis1=dis1.reshape(T1, P).T.copy(),
                  inv0=cd["inv0"].reshape(1, T0 * P).astype(BF16),
                  inv1=inv1.reshape(1, T1 * P).astype(BF16),
                  mctT=mctT.astype(BF16), slot1_node=slot1_node)

    shared = dict(
        w0=np.ascontiguousarray(W0, np.float32).astype(BF16),
        w1=np.ascontiguousarray(W1, np.float32).reshape(2, P, HID).astype(BF16),
        b0r=np.ascontiguousarray(b0, np.float32).reshape(1, HID).astype(BF16),
        b1r=np.ascontiguousarray(b1, np.float32).reshape(1, HID).astype(BF16),
        ident=np.eye(P, dtype=np.float32).astype(BF16),
    )
    zero_bias = bool(np.all(np.asarray(b0) == 0) and np.all(np.asarray(b1) == 0))
    meta = dict(N=N, IN=IN, HID=HID, G=G, SH=SH, T0=T0, T1=T1,
                NW0=NW0, NW1=NW1, C0=C0, C1=C1, zero_bias=zero_bias,
                D0=D0t, D1=D1t, P1=P1t, Q1=Q1t)
    fin = dict(W2=np.asarray(W2, np.float32), b2=np.asarray(b2, np.float32))
    return meta, shared, cores, fin


# ---------------------------------------------------------------------------
# Pure-numpy emulation of the device program (validation / debugging)
# ---------------------------------------------------------------------------

def _emulate(meta, shared, cores, fin):
    T0, T1, HID, IN, G = (meta[k] for k in ("T0", "T1", "HID", "IN", "G"))
    NW0, NW1 = meta["NW0"], meta["NW1"]
    D0, D1, P1, Q1 = meta["D0"], meta["D1"], meta["P1"], meta["Q1"]
    _, calls0, colbase0, C0 = _schedule(D0, NW0, GCOLS0, CCAP0)
    _, calls1, colbase1, C1 = _schedule(D1, NW1)
    w0 = shared["w0"].astype(np.float32)
    w1 = shared["w1"].astype(np.float32).reshape(2 * P, HID)
    b0 = shared["b0r"].astype(np.float32)[0]
    b1 = shared["b1r"].astype(np.float32)[0]

    def col_windows(calls, C):
        cw = np.zeros(C, np.int64)
        for w, n, off in calls:
            cw[off:off + n] = w
        return cw

    cw0 = col_windows(calls0, C0)
    cw1 = col_windows(calls1, C1)

    Y = np.zeros((G, HID), np.float32)
    for cd in cores:
        xh = cd["xh"].astype(np.float32)
        rows = cw0.repeat(P) * WSTR + cd["flat0"]
        gat = xh[rows].reshape(C0, P, 4 * IN)
        u0 = cd["xselfT"].astype(np.float32).T.copy()   # [T0*P, IN]
        for t in range(T0):
            for w in range(NW0):
                cb = colbase0[t][w]
                for j in range(D0[t][w]):
                    npl = 4 if j < Q1[t][w] else (2 if j < P1[t][w] else 1)
                    for pl in range(npl):
                        u0[t * P:(t + 1) * P] += \
                            gat[cb + j][:, pl * IN:(pl + 1) * IN]
        u0 = u0.astype(BF16).astype(np.float32)
        inv0 = cd["inv0"].astype(np.float32)[0]
        pre = u0 @ w0 + inv0[:, None] * b0[None, :]
        d0 = cd["dis0"].T.reshape(-1)
        h1 = np.maximum(d0[:, None] * pre, 0).astype(FP8).astype(np.float32)
        # place h1 into windowed layout (grouped rows)
        h1w = np.zeros((NW1 * WSTR, HID), np.float32)
        h1w[_h1row(np.arange(T0 * P))] = h1

        rows1 = cw1.repeat(P) * WSTR + cd["flat1"]
        gat1 = h1w[rows1].reshape(C1, P, HID)
        u1 = np.zeros((T1 * P, HID), np.float32)
        for t in range(T1):
            for w in range(NW1):
                cb = colbase1[t][w]
                for j in range(D1[t][w]):
                    u1[t * P:(t + 1) * P] += gat1[cb + j]
        u1 = u1.astype(BF16).astype(np.float32)
        inv1 = cd["inv1"].astype(np.float32)[0]
        pre1 = u1 @ w1 + inv1[:, None] * b1[None, :]
        d1 = cd["dis1"].T.reshape(-1)
        h2 = np.maximum(d1[:, None] * pre1, 0).astype(BF16).astype(np.float32)
        mctT = cd["mctT"].astype(np.float32)
        for t in range(T1):
            Y += mctT[:, t * G:(t + 1) * G].T @ h2[t * P:(t + 1) * P]
    return Y @ fin["W2"] + fin["b2"]


# ---------------------------------------------------------------------------
# Bass device program
# ---------------------------------------------------------------------------

def _build(meta):
    import concourse.bass as bass
    import concourse.mybir as mybir
    import concourse.tile as tile
    from concourse import bacc, library_config
    from concourse.tile_rust import add_dep_helper

    F32, I16 = mybir.dt.float32, mybir.dt.int16
    BF = mybir.dt.bfloat16
    F8 = mybir.dt.float8e4
    RELU = mybir.ActivationFunctionType.Relu

    IN, HID, G = meta["IN"], meta["HID"], meta["G"]
    T0, T1 = meta["T0"], meta["T1"]
    NW0, NW1 = meta["NW0"], meta["NW1"]
    C0, C1 = meta["C0"], meta["C1"]
    D0, D1, P1, Q1 = meta["D0"], meta["D1"], meta["P1"], meta["Q1"]
    ZB = meta["zero_bias"]
    groups0, calls0, colbase0, _ = _schedule(D0, NW0, GCOLS0, CCAP0)
    groups1, calls1, colbase1, _ = _schedule(D1, NW1)
    c2c0 = _call_of_col(calls0)
    c2c1 = _call_of_col(calls1)

    def _max_calls_per_group(groups, colbase, D, NW, c2c):
        mx = 1
        for (t0, t1) in groups:
            ids = set()
            for tt in range(t0, t1):
                for w in range(NW):
                    for j in range(D[tt][w]):
                        ids.add(c2c[colbase[tt][w] + j][0])
            mx = max(mx, len(ids))
        return mx

    gbufs0 = _max_calls_per_group(groups0, colbase0, D0, NW0, c2c0) + 4
    gbufs1 = _max_calls_per_group(groups1, colbase1, D1, NW1, c2c1) + 4

    nc = bacc.Bacc("TRN2", target_bir_lowering=False, debug=False,
                   num_devices=NCORES)

    t_xh = nc.dram_tensor("xh", [NW0 * WSTR, 4 * IN], F8,
                          kind="ExternalInput")
    t_xst = nc.dram_tensor("xselfT", [IN, T0 * P], F8, kind="ExternalInput")
    t_idx0 = nc.dram_tensor("idx0", [P, C0 * 8], I16, kind="ExternalInput")
    t_idx1 = nc.dram_tensor("idx1", [P, C1 * 8], I16, kind="ExternalInput")
    t_dis0 = nc.dram_tensor("dis0", [P, T0], F32, kind="ExternalInput")
    t_dis1 = nc.dram_tensor("dis1", [P, T1], F32, kind="ExternalInput")
    t_inv0 = nc.dram_tensor("inv0", [1, T0 * P], BF, kind="ExternalInput")
    t_inv1 = nc.dram_tensor("inv1", [1, T1 * P], BF, kind="ExternalInput")
    t_w0 = nc.dram_tensor("w0", [IN, HID], BF, kind="ExternalInput")
    t_w1 = nc.dram_tensor("w1", [2, P, HID], BF, kind="ExternalInput")
    t_b0 = nc.dram_tensor("b0r", [1, HID], BF, kind="ExternalInput")
    t_b1 = nc.dram_tensor("b1r", [1, HID], BF, kind="ExternalInput")
    t_mct = nc.dram_tensor("mctT", [P, T1 * G], BF, kind="ExternalInput")
    t_id = nc.dram_tensor("ident", [P, P], BF, kind="ExternalInput")
    t_out = nc.dram_tensor("outp", [G, HID], F32, kind="ExternalOutput")

    with tile.TileContext(nc) as tc:
        with (
            tc.tile_pool(name="const", bufs=1) as cpool,
            tc.tile_pool(name="ut", bufs=4) as upool,
            tc.tile_pool(name="stage", bufs=4) as spool,
            tc.tile_pool(name="h2res", bufs=10) as h2pool,
            tc.tile_pool(name="aggps", bufs=3, space="PSUM") as apool,
            tc.tile_pool(name="agg1ps", bufs=2, space="PSUM") as a1pool,
            tc.tile_pool(name="preps", bufs=2, space="PSUM") as ppool,
            tc.tile_pool(name="outps", bufs=1, space="PSUM") as opool,
            tc.tile_pool(name="dram", bufs=1, space="DRAM") as dpool,
        ):
            lib = nc.gpsimd.load_library(library_config.mlp)

            def cload(t, shape, dt):
                s = cpool.tile(shape, dt, tag=t.name)
                nc.sync.dma_start(s[:], t[:])
                return s

            ident = cload(t_id, [P, P], BF)
            ident8 = cpool.tile([P, P], F8, tag="ident8")
            nc.vector.tensor_copy(ident8[:], ident[:])
            w0 = cload(t_w0, [IN, HID], BF)
            w1 = cpool.tile([P, 2, HID], BF, tag="w1")
            nc.sync.dma_start(w1[:], t_w1[:].rearrange("j p h -> p j h"))
            if not ZB:
                b0r = cload(t_b0, [1, HID], BF)
                b1r = cload(t_b1, [1, HID], BF)
            dis0 = cload(t_dis0, [P, T0], F32)
            dis1 = cload(t_dis1, [P, T1], F32)
            mt = cload(t_mct, [P, T1 * G], BF)

            h1h = dpool.tile([NW1 * WSTR, HID], F8)

            h1h_writes = []
            zt = spool.tile([P, HID], F8, tag="zrow")
            nc.vector.memset(zt[:], 0.0)
            for w in range(NW1):
                h1h_writes.append(nc.sync.dma_start(
                    h1h[w * WSTR:w * WSTR + 1, :], zt[0:1, :]))

            def slot_row(s):
                return (s // WIN1) * WSTR + 1 + (s % WIN1)

            def do_gathers(t_src_ap, idx_sb, calls, grp_calls, buf_pool,
                           ccap_elem, elem, deps, dt=BF):
                out = {}
                for ci in grp_calls:
                    w, ncols, off = calls[ci]
                    gt = buf_pool.tile([P, ccap_elem], dt, tag="g")
                    gi = nc.gpsimd.dma_gather(
                        gt[:, :ncols * elem].rearrange(
                            "p (j d) -> p j d", j=ncols),
                        t_src_ap(w),
                        idx_sb[:, off * 8:(off + ncols) * 8],
                        ncols * P, ncols * P, elem, single_packet=False)
                    add_dep_helper(gi.ins, lib.ins, True, "gather after lib")
                    dl = deps(w) if callable(deps) else deps
                    for d in dl:
                        add_dep_helper(gi.ins, d.ins, True, "gather after src")
                    out[ci] = gt
                return out

            def grp_call_ids(t0, t1, colbase, D, NW, c2c):
                ids = set()
                for tt in range(t0, t1):
                    for w in range(NW):
                        for j in range(D[tt][w]):
                            ids.add(c2c[colbase[tt][w] + j][0])
                return sorted(ids)

            # ---------------- Layer 0 ----------------
            with tc.tile_pool(name="idx0p", bufs=1) as ipool0, \
                 tc.tile_pool(name="g0", bufs=gbufs0) as gpool0, \
                 tc.tile_pool(name="xs0", bufs=3) as xspool:
                idx0 = ipool0.tile([P, C0 * 8], I16, tag="idx0")
                nc.sync.dma_start(idx0[:], t_idx0[:])

                bank = None
                uT4 = None
                stage_t = None
                for (t0g, t1g) in groups0:
                    ids = grp_call_ids(t0g, t1g, colbase0, D0, NW0, c2c0)
                    bufs = do_gathers(
                        lambda w: t_xh[w * WSTR:(w + 1) * WSTR, :],
                        idx0, calls0, ids, gpool0, CCAP0 * 4 * IN, 4 * IN,
                        [], dt=F8)
                    # dense self-loop chunk for this group's tiles
                    xst = xspool.tile([P, (t1g - t0g) * P], F8, tag="xs")
                    nc.sync.dma_start(xst[:],
                                      t_xst[:, t0g * P:t1g * P])
                    for t in range(t0g, t1g):
                        q = t % 4
                        if q == 0:
                            bank = apool.tile([P, 4 * P], mybir.dt.float32,
                                              tag="aggps", space="PSUM")
                        ups = bank[:, q * P:(q + 1) * P]
                        nd = 0
                        for w in range(NW0):
                            for j in range(D0[t][w]):
                                nd += (4 if j < Q1[t][w]
                                       else (2 if j < P1[t][w] else 1))
                        # self contribution: ups = I @ xselfT_t
                        nc.tensor.matmul(
                            ups, lhsT=ident8[:],
                            rhs=xst[:, (t - t0g) * P:(t - t0g + 1) * P],
                            start=True, stop=(nd == 0))
                        k = 0
                        for w in range(NW0):
                            for j in range(D0[t][w]):
                                ci, lc = c2c0[colbase0[t][w] + j]
                                gt = bufs[ci]
                                nplanes = (4 if j < Q1[t][w]
                                           else (2 if j < P1[t][w] else 1))
                                for pl in range(nplanes):
                                    nc.tensor.matmul(
                                        ups,
                                        lhsT=gt[:, lc * 4 * IN + pl * IN:
                                                lc * 4 * IN + (pl + 1) * IN],
                                        rhs=ident8[:], start=False,
                                        stop=(k == nd - 1))
                                    k += 1
                        if q == 3:
                            uT4 = upool.tile([P, 4 * P], BF, tag="ut")
                            nc.vector.tensor_copy(uT4[:], bank[:])
                        # defer pre/act until uT4 is ready
                        if q == 3:
                            for tq in range(t - 3, t + 1):
                                qq = tq % 4
                                pre = ppool.tile([P, HID], mybir.dt.float32,
                                                 tag="preps", space="PSUM")
                                if ZB:
                                    nc.tensor.matmul(
                                        pre[:],
                                        lhsT=uT4[:, qq * P:(qq + 1) * P],
                                        rhs=w0[:], start=True, stop=True)
                                else:
                                    sd = upool.tile([1, P], BF, tag="seed")
                                    nc.sync.dma_start(
                                        sd[:], t_inv0[:, tq * P:(tq + 1) * P])
                                    nc.tensor.matmul(pre[:], lhsT=sd[:],
                                                     rhs=b0r[:],
                                                     start=True, stop=False)
                                    nc.tensor.matmul(
                                        pre[:],
                                        lhsT=uT4[:, qq * P:(qq + 1) * P],
                                        rhs=w0[:], start=False, stop=True)
                                if qq == 0:
                                    stage_t = spool.tile([P, 4 * HID], F8,
                                                         tag="h1stage")
                                nc.scalar.activation(
                                    stage_t[:, qq * HID:(qq + 1) * HID],
                                    pre[:], RELU, bias=0.0,
                                    scale=dis0[:, tq:tq + 1])
                                if qq == 3:
                                    r0 = slot_row((tq - 3) * P)
                                    h1h_writes.append(nc.sync.dma_start(
                                        h1h[r0:r0 + 4 * P, :].rearrange(
                                            "(p j) h -> p j h", p=P),
                                        stage_t[:].rearrange(
                                            "p (j h) -> p j h", j=4)))

            # ---------------- Layer 1 ----------------
            # writes [0, NW1) are the zero rows; block write i covers slots
            # [512i, 512i+512) -> window 512i // WIN1
            h1h_deps = {
                w: [h1h_writes[w]] + [wr for i, wr in
                                      enumerate(h1h_writes[NW1:])
                                      if (512 * i) // WIN1 == w]
                for w in range(NW1)
            }

            with tc.tile_pool(name="idx1p", bufs=1) as ipool1, \
                 tc.tile_pool(name="g1", bufs=gbufs1) as gpool1:
                idx1 = ipool1.tile([P, C1 * 8], I16, tag="idx1")
                nc.sync.dma_start(idx1[:], t_idx1[:])

                bank = None
                h2_tiles = {}
                pooled = 0
                ops = opool.tile([G, HID], mybir.dt.float32,
                                 tag="outps", space="PSUM")

                def pool_flush(upto):
                    nonlocal pooled
                    while pooled < upto:
                        tp = pooled
                        ht = h2_tiles.pop(tp)
                        nc.tensor.matmul(
                            ops[:], lhsT=mt[:, tp * G:(tp + 1) * G],
                            rhs=ht[:, (tp % 2) * HID:(tp % 2 + 1) * HID],
                            start=(tp == 0), stop=(tp == T1 - 1))
                        pooled += 1

                for (t0g, t1g) in groups1:
                    ids = grp_call_ids(t0g, t1g, colbase1, D1, NW1, c2c1)
                    bufs = do_gathers(
                        lambda w: h1h[w * WSTR:(w + 1) * WSTR, :],
                        idx1, calls1, ids, gpool1, CCAP * HID, HID,
                        lambda w: h1h_deps[w], dt=F8)
                    for t in range(t0g, t1g):
                        q = t % 2
                        if q == 0:
                            bank = a1pool.tile([P, 4 * P], mybir.dt.float32,
                                               tag="agg1ps", space="PSUM")
                        u0ps = bank[:, q * HID:q * HID + P]
                        u1ps = bank[:, q * HID + P:(q + 1) * HID]
                        nd = sum(D1[t])
                        if nd == 0:
                            nc.tensor.matmul(u0ps, lhsT=ident[:],
                                             rhs=ident[:], start=True,
                                             stop=True)
                            nc.tensor.matmul(u1ps, lhsT=ident[:],
                                             rhs=ident[:], start=True,
                                             stop=True)
                        for half, ups in ((0, u0ps), (1, u1ps)):
                            k = 0
                            for w in range(NW1):
                                for j in range(D1[t][w]):
                                    ci, lc = c2c1[colbase1[t][w] + j]
                                    gt = bufs[ci]
                                    nc.tensor.matmul(
                                        ups,
                                        lhsT=gt[:, lc * HID + half * P:
                                                lc * HID + (half + 1) * P],
                                        rhs=ident8[:], start=(k == 0),
                                        stop=(k == nd - 1))
                                    k += 1
                        if q == 1:
                            uT4 = upool.tile([P, 4 * P], BF, tag="ut")
                            nc.vector.tensor_copy(uT4[:], bank[:])
                            for tq in (t - 1, t):
                                qq = tq % 2
                                pre = ppool.tile([P, HID], mybir.dt.float32,
                                                 tag="preps", space="PSUM")
                                if not ZB:
                                    sd = upool.tile([1, P], BF, tag="seed")
                                    nc.sync.dma_start(
                                        sd[:], t_inv1[:, tq * P:(tq + 1) * P])
                                    nc.tensor.matmul(pre[:], lhsT=sd[:],
                                                     rhs=b1r[:],
                                                     start=True, stop=False)
                                nc.tensor.matmul(
                                    pre[:],
                                    lhsT=uT4[:, qq * HID:qq * HID + P],
                                    rhs=w1[:, 0, :], start=ZB, stop=False)
                                nc.tensor.matmul(
                                    pre[:],
                                    lhsT=uT4[:, qq * HID + P:(qq + 1) * HID],
                                    rhs=w1[:, 1, :], start=False, stop=True)
                                if tq % 2 == 0:
                                    h2_t = h2pool.tile([P, 2 * HID], BF,
                                                       tag="h2s")
                                    h2_tiles[tq] = h2_tiles[tq + 1] = h2_t
                                nc.scalar.activation(
                                    h2_tiles[tq][:, (tq % 2) * HID:
                                                 (tq % 2 + 1) * HID],
                                    pre[:], RELU, bias=0.0,
                                    scale=dis1[:, tq:tq + 1])
                                # pooling matmuls lag 16 tiles behind so
                                # they never wait on a fresh activation
                                pool_flush(max(0, tq - 15))

                pool_flush(T1)
                osb = spool.tile([G, HID], mybir.dt.float32, tag="osb")
                nc.vector.tensor_copy(osb[:], ops[:])
                nc.sync.dma_start(t_out[:], osb[:])

    nc.compile()
    return nc


# ---------------------------------------------------------------------------
# Entry point
# ---------------------------------------------------------------------------

_cache = {}


def _get_nc(meta):
    key = hashlib.sha1(repr(sorted(meta.items())).encode()).hexdigest()
    if key not in _cache:
        _cache[key] = _build(meta)
    return _cache[key]


def _in_maps(shared, cores):
    maps = []
    for cd in cores:
        m = dict(shared)
        m.update({k: cd[k] for k in
                  ("xh", "xselfT", "idx0", "idx1", "dis0", "dis1",
                   "inv0", "inv1", "mctT")})
        maps.append(m)
    return maps


def _run_device(meta, shared, cores):
    from concourse.bass_utils import run_bass_kernel_spmd
    nc = _get_nc(meta)
    res = run_bass_kernel_spmd(nc, _in_maps(shared, cores),
                               core_ids=list(range(NCORES)))
    return [r["outp"] for r in res.results]


def kernel(**inputs):
    meta, shared, cores, fin = _preprocess(**inputs)
    outs = _run_device(meta, shared, cores)
    Y = np.sum(np.stack(outs), axis=0, dtype=np.float32)
    out = Y @ fin["W2"] + fin["b2"]
    return out.astype(np.float32)


def profile_run(meta, shared, cores, trace_cores=None):
    """Profiled exec time in ns: NTFF trace when available, else the
    instruction-cost-model timeline simulation of the compiled program."""
    from concourse.bass_utils import run_bass_kernel_spmd
    nc = _get_nc(meta)
    try:
        res = run_bass_kernel_spmd(nc, _in_maps(shared, cores),
                                   core_ids=list(range(NCORES)), trace=True,
                                   trace_cores=trace_cores)
        if res.exec_time_ns is not None:
            print("profile:", res.instructions_and_trace[1]
                  if res.instructions_and_trace else None)
            return res.exec_time_ns
    except Exception as e:
        print(f"NTFF trace unavailable ({type(e).__name__}); "
              "using cost-model timeline")
    from concourse.timeline_sim import TimelineSim
    ts = TimelineSim(nc, trace=False)
    ts.simulate()
    return int(ts.time)


# revision 81
# speedup vs baseline: 1.0435x; 1.0271x over previous
"""GCN (3-layer GCNConv + global mean pool) on 8 Trainium2 NeuronCores.

Math: with S = adjacency+self-loops and D = diag(1/sqrt(deg)),
    conv(h) = relu(D S D h W + b)
and the diagonal scalings commute with the dense W, so each layer is an
UNWEIGHTED gather-sum of pre-scaled rows plus a dense matmul.  The final
conv + mean-pool collapse into a dense host-precomputed pooling matrix:
    out = (Mhat @ h2) @ W2 + b2,   Mhat = diag(1/cnt) S_pool A_norm.

Sharding: nodes dst-partitioned across 8 cores; layer 0 is recomputed on
each core's halo (src nodes of its incident edges) so cores never
communicate.  Optimizations over the plain halo design:
  * per-core xh holds only the deduplicated layer-0 edge sources (3
    int16-index windows, placed by local out-degree), so gather columns
    carry far less padding than a shared node-id-ordered table;
  * self-loop contributions stream in as a dense pre-transposed
    [feat, slot] table (bytes-bound DMA) and enter PSUM via one
    identity matmul per tile instead of per-slot gather descriptors;
  * slot schedules are budgeted per window-count pattern across cores so
    all 8 cores share one compiled program with an exact column profile;
  * PSUM aggregation packs 4 tiles per bank, copied out in one DVE op;
  * h2 never round-trips DRAM: layer-1 outputs stay in SBUF and feed the
    pooling matmuls directly, with the pooling matrix staged in a
    partition-tiled layout (large DMA descriptors).
"""

import hashlib

import numpy as np
import ml_dtypes

P = 128
NCORES = 8
WSTR = 32768      # physical window stride (rows); row w*WSTR is all-zero
WIN0 = 32767      # usable rows per layer-0 source window
WIN1 = 32256      # usable slots per layer-1 source window (512-aligned)
GCOLS = 96        # gather column budget per tile-group (layer 1)
CCAP = 32         # max columns per dma_gather call (layer 1)
GCOLS0 = 64       # layer-0 columns are 512B pair-reads: smaller groups
CCAP0 = 16

BF16 = ml_dtypes.bfloat16
FP8 = ml_dtypes.float8_e4m3


def _h1row(s):
    """Slot -> h1h row (grouped so that one partition's 4 rows of a write
    block are contiguous: 1KB fp8 descriptors).  WIN1 % 512 == 0 keeps
    512-slot blocks window-aligned."""
    w = s // WIN1
    loc = s - w * WIN1
    b = loc // 512
    p = s % P
    j = (s // P) % 4
    return w * WSTR + 1 + b * 512 + p * 4 + j


# ---------------------------------------------------------------------------
# Shared schedule derivation (host + builder + emulator all use this)
# ---------------------------------------------------------------------------

def _schedule(D, NW, gcols=GCOLS, ccap=CCAP):
    """D: [T][NW] column counts.  Returns (groups, calls, colbase, Ctot).

    groups: list of (t0, t1) tile ranges with total columns <= gcols.
    calls: list of (w, ncols, col_off) in global column order; a call's
        columns are consecutive.  Global column order: per group, per
        window (ascending), per tile (ascending), per j.
    colbase: [T][NW] global column offset of (t, w)'s first column.
    """
    T = len(D)
    groups = []
    t = 0
    while t < T:
        # taper: the last tiles get half-size groups so the compute tail
        # after the final gather stays short
        lim = gcols if t < T - 24 else max(gcols // 4, 8)
        tot = sum(D[t])
        t1 = t + 1
        while t1 < T and tot + sum(D[t1]) <= lim:
            tot += sum(D[t1])
            t1 += 1
        groups.append((t, t1))
        t = t1
    colbase = [[0] * NW for _ in range(T)]
    calls = []
    off = 0
    for (t0, t1) in groups:
        for w in range(NW):
            cols = 0
            for tt in range(t0, t1):
                colbase[tt][w] = off + cols
                cols += D[tt][w]
            c0 = 0
            while c0 < cols:
                n = min(ccap, cols - c0)
                calls.append((w, n, off + c0))
                c0 += n
            off += cols
    return groups, calls, colbase, off


def _call_of_col(calls):
    """Map global column -> (call_id, local_col)."""
    m = {}
    for ci, (w, n, off) in enumerate(calls):
        for j in range(n):
            m[off + j] = (ci, j)
    return m


# ---------------------------------------------------------------------------
# Host preprocessing
# ---------------------------------------------------------------------------

def _budget_slots(pats_per_core, round_slots):
    """Shared pattern budget: every distinct window-count vector gets
    max-over-cores slots at a FIXED (lexicographically ordered) position.
    Returns (slotpat [T*P, NW], class ranges dict pat->slice, T)."""
    cat = np.concatenate(pats_per_core)
    upat, inv = np.unique(cat, axis=0, return_inverse=True)
    npat = len(upat)
    counts = np.zeros((len(pats_per_core), npat), np.int64)
    off = 0
    for c, p in enumerate(pats_per_core):
        counts[c] = np.bincount(inv[off:off + len(p)], minlength=npat)
        off += len(p)
    budget = counts.max(axis=0)
    lex = np.lexsort(upat.T[::-1])
    tot = int(budget.sum())
    T = -(-tot // round_slots) * (round_slots // P)
    slotpat = np.zeros((T * P, upat.shape[1]), np.int64)
    starts = np.zeros(npat, np.int64)
    pos = T * P - tot          # pads first
    for pi in lex:
        slotpat[pos:pos + budget[pi]] = upat[pi]
        starts[pi] = pos
        pos += budget[pi]
    return slotpat, upat, starts, T


def _fill_idx(slots_of_edges, rows_of_edges, w_of_edges, jj, colbase, D,
              Ctot, NW):
    """Build the flat int16 gather index list [Ctot*128] (0 = window zero
    row), then wrap for dma_gather: [128, Ctot*8].

    slots_of_edges: slot id per edge; rows_of_edges: +1-based row within
    window; w_of_edges: window id; jj: stable per-(slot,window) rank.
    """
    flat = np.zeros(Ctot * P, np.int16)
    tt = slots_of_edges // P
    pp = slots_of_edges % P
    cb = np.asarray(colbase, np.int64)
    col = cb[tt, w_of_edges] + jj
    flat[col * P + pp] = rows_of_edges.astype(np.int16)
    wrapped = np.tile(flat.reshape(-1, 16).T, (8, 1))
    return flat, wrapped


def _rank_within(a, b):
    """Stable rank of each element within its (a, b) group (a, b int arrays,
    pre-sorted arbitrary order)."""
    o = np.lexsort((b, a))
    sa, sb = a[o], b[o]
    change = np.ones(len(o), bool)
    change[1:] = (sa[1:] != sa[:-1]) | (sb[1:] != sb[:-1])
    starts = np.nonzero(change)[0]
    gid = np.cumsum(change) - 1
    rk = np.arange(len(o)) - starts[gid]
    out = np.empty(len(o), np.int64)
    out[o] = rk
    return out


def _preprocess(x, edge_index, batch, num_graphs, W0, b0, W1, b1, W2, b2):
    x = np.asarray(x, np.float32)
    N, IN = x.shape
    HID = W0.shape[1]
    G = int(num_graphs)
    SH = N // NCORES
    src = np.asarray(edge_index[0], np.int64)
    dst = np.asarray(edge_index[1], np.int64)
    batch = np.asarray(batch, np.int64)

    degi = np.bincount(dst, minlength=N) + 1          # + self-loop
    dis = (1.0 / np.sqrt(degi.astype(np.float64))).astype(np.float32)
    invd = np.sqrt(degi.astype(np.float64)).astype(np.float32)

    order = np.argsort(dst, kind="stable")
    s_sorted = src[order]
    indptr = np.searchsorted(dst, np.arange(N + 1), sorter=order)

    xhat = x * dis[:, None]

    # dense pooling matrix Mhat = diag(1/cnt) @ S_pool @ A_norm  [G, N]
    cnt = np.bincount(batch, minlength=G).astype(np.float64)
    cntc = np.maximum(cnt, 1.0)
    bd = batch[dst]
    w_ = dis[dst].astype(np.float64) * dis[src] / cntc[bd]
    M = np.bincount(bd * N + src, weights=w_, minlength=G * N)
    w2_ = dis.astype(np.float64) ** 2 / cntc[batch]
    M += np.bincount(batch * N + np.arange(N), weights=w2_, minlength=G * N)
    Mhat = M.reshape(G, N).astype(np.float32)

    # ---------------- per-core layer-0 structure (with pair matching) ----
    core_l0 = []
    pats0 = []
    for c in range(NCORES):
        own = np.arange(c * SH, (c + 1) * SH)
        esrc = s_sorted[indptr[c * SH]:indptr[(c + 1) * SH]]
        halo = np.unique(np.concatenate([esrc, own]))
        deg = indptr[halo + 1] - indptr[halo]          # ext in-deg
        E0 = int(deg.sum())
        rep = np.repeat(np.arange(len(halo)), deg)
        ei = np.repeat(indptr[halo], deg) + (
            np.arange(E0) - np.repeat(np.cumsum(deg) - deg, deg))
        l0src = s_sorted[ei]
        uniq, l0inv = np.unique(l0src, return_inverse=True)
        U = len(uniq)

        # Greedy pair packing: a 512-byte descriptor covers two of a dst's
        # sources stored adjacently; pair rows may share a source (rows are
        # duplicated as needed), so coverage is per-dst set packing.
        o = np.lexsort((l0inv, rep))
        rs, us = rep[o], l0inv[o]
        dgs = np.bincount(rs, minlength=len(halo))
        offs = np.concatenate([[0], np.cumsum(dgs)])
        occ_d, occ_a, occ_b = [], [], []
        for k1 in range(int(dgs.max())):
            for k2 in range(k1 + 1, int(dgs.max())):
                sel = dgs > k2
                i0 = offs[:-1][sel]
                a, b = us[i0 + k1], us[i0 + k2]
                ok = a != b
                occ_d.append(np.nonzero(sel)[0][ok])
                occ_a.append(np.minimum(a[ok], b[ok]))
                occ_b.append(np.maximum(a[ok], b[ok]))
        occ_d = np.concatenate(occ_d)
        occ_a = np.concatenate(occ_a)
        occ_b = np.concatenate(occ_b)
        okey = occ_a * U + occ_b
        uk, kinv, ucnt = np.unique(okey, return_inverse=True,
                                   return_counts=True)
        order_occ = np.lexsort((kinv, -ucnt[kinv]))

        dk = rep * U + l0inv
        udk, cnt_du = np.unique(dk, return_counts=True)
        avail = cnt_du.copy()
        ia_occ = np.searchsorted(udk, occ_d * U + occ_a)
        ib_occ = np.searchsorted(udk, occ_d * U + occ_b)

        rowid_of_key = {}
        prow_a, prow_b = [], []
        use_d, use_row, use_k = [], [], []
        for i in order_occ:
            ka = avail[ia_occ[i]]
            kb = avail[ib_occ[i]]
            k = ka if ka < kb else kb
            if k <= 0:
                continue
            rk = okey[i]
            rid = rowid_of_key.get(rk)
            if rid is None:
                rid = len(prow_a)
                rowid_of_key[rk] = rid
                prow_a.append(occ_a[i])
                prow_b.append(occ_b[i])
            avail[ia_occ[i]] = ka - k
            avail[ib_occ[i]] = kb - k
            use_d.append(occ_d[i])
            use_row.append(rid)
            use_k.append(k)
        NPAIRROW = len(prow_a)
        prow_a = np.array(prow_a, np.int64)
        prow_b = np.array(prow_b, np.int64)
        use_d = np.array(use_d, np.int64)
        use_row = np.array(use_row, np.int64)
        use_k = np.array(use_k, np.int64)

        # tier 2: merge two pair-uses of one dst into a quad descriptor (a
        # 512B fp8 row holds 4 sources)
        pu_d = np.repeat(use_d, use_k)
        pu_row = np.repeat(use_row, use_k)
        o2 = np.lexsort((pu_row, pu_d))
        pu_d, pu_row = pu_d[o2], pu_row[o2]
        runc = np.ones(len(pu_d), bool)
        runc[1:] = pu_d[1:] != pu_d[:-1]
        rstart = np.nonzero(runc)[0]
        rix = np.arange(len(pu_d)) - rstart[np.cumsum(runc) - 1]
        is_even = (rix % 2 == 0)
        has_next = np.zeros(len(pu_d), bool)
        if len(pu_d) > 1:
            has_next[:-1] = is_even[:-1] & (pu_d[:-1] == pu_d[1:])
        qa_idx = np.nonzero(has_next)[0]
        q_d = pu_d[qa_idx]
        q_r1 = pu_row[qa_idx]
        q_r2 = pu_row[qa_idx + 1]
        qlo = np.minimum(q_r1, q_r2)
        qhi = np.maximum(q_r1, q_r2)
        qkey = qlo * NPAIRROW + qhi if NPAIRROW else qlo
        uqk, qinv = np.unique(qkey, return_inverse=True)
        NQUAD = len(uqk)
        quad_p1 = uqk // max(NPAIRROW, 1)
        quad_p2 = uqk % max(NPAIRROW, 1)
        # leftover pair-uses (not merged)
        merged = np.zeros(len(pu_d), bool)
        merged[qa_idx] = True
        merged[qa_idx + 1] = True
        lp_d = pu_d[~merged]
        lp_row = pu_row[~merged]
        # pair rows still referenced directly
        pair_used = np.zeros(NPAIRROW, bool)
        pair_used[lp_row] = True
        pmap = np.full(NPAIRROW, -1, np.int64)
        pmap[pair_used] = np.arange(int(pair_used.sum()))
        NPAIR = int(pair_used.sum())

        # single uses = leftover availability per (dst, src)
        dd, uu = divmod(udk, U)
        sing_du = avail
        single_src_tot = np.bincount(uu, weights=sing_du.astype(np.float64),
                                     minlength=U).astype(np.int64)
        srow_ids = np.nonzero(single_src_tot > 0)[0]
        smap = np.full(U, -1, np.int64)
        smap[srow_ids] = np.arange(len(srow_ids))
        nrows = NQUAD + NPAIR + len(srow_ids)
        quad_pop = np.bincount(qinv, minlength=NQUAD).astype(np.int64)
        pair_pop = np.bincount(pmap[lp_row], minlength=NPAIR).astype(np.int64)
        pop = np.concatenate([quad_pop, pair_pop, single_src_tot[srow_ids]])
        rank = np.empty(nrows, np.int64)
        rank[np.argsort(-pop, kind="stable")] = np.arange(nrows)
        row_w = rank // WIN0
        row_loc = rank % WIN0 + 1                      # +1: row 0 is zeros
        NW0c = int(row_w.max()) + 1 if nrows else 1

        # flat descriptor list: (halo idx, window, loc, nplanes)
        vv = sing_du > 0
        d_i = np.concatenate([q_d, lp_d, np.repeat(dd[vv], sing_du[vv])])
        d_row = np.concatenate([rank[qinv],
                                rank[NQUAD + pmap[lp_row]],
                                np.repeat(rank[NQUAD + NPAIR +
                                               smap[uu[vv]]],
                                          sing_du[vv])])
        d_np = np.concatenate([np.full(len(q_d), 4, np.int64),
                               np.full(len(lp_d), 2, np.int64),
                               np.ones(int(sing_du[vv].sum()), np.int64)])
        d_w = d_row // WIN0
        d_loc = d_row % WIN0 + 1

        pat = np.zeros((len(halo), 3), np.int64)
        np.add.at(pat, (d_i, d_w), 1)
        qpat = np.zeros((len(halo), 3), np.int64)
        np.add.at(qpat, (d_i[d_np == 4], d_w[d_np == 4]), 1)
        ppat = np.zeros((len(halo), 3), np.int64)
        np.add.at(ppat, (d_i[d_np == 2], d_w[d_np == 2]), 1)

        core_l0.append(dict(own=own, halo=halo, uniq=uniq, NW0c=NW0c,
                            d_i=d_i, d_w=d_w, d_loc=d_loc, d_np=d_np,
                            qpat=qpat, ppat=ppat,
                            prow_a=prow_a, prow_b=prow_b,
                            quad_p1=quad_p1, quad_p2=quad_p2,
                            pair_used=pair_used, pmap=pmap,
                            srow_ids=srow_ids, row_w=row_w, row_loc=row_loc,
                            NQUAD=NQUAD, NPAIR=NPAIR))
        pats0.append(pat)

    NW0 = max(d["NW0c"] for d in core_l0)
    slotpat0, upat0, starts0, T0 = _budget_slots(pats0, 512)
    assert T0 * P <= 2 * WIN1, "layer-1 source exceeds two windows"
    D0 = slotpat0.reshape(T0, P, 3).max(axis=1)
    D0t = tuple(tuple(int(v) for v in row) for row in D0)
    _, calls0, colbase0, C0 = _schedule(D0t, NW0, GCOLS0, CCAP0)

    key0 = {tuple(p): i for i, p in enumerate(upat0)}

    # ---------------- per-core slot assignment + idx0 ----------------
    cores = []
    pats1 = []
    l1_parts = []
    P1_cores = []
    Q1_cores = []
    for c in range(NCORES):
        d = core_l0[c]
        halo, pat = d["halo"], pats0[c]
        cls = np.array([key0[tuple(p)] for p in pat], np.int64)
        # within a class, cluster similar quad/pair-count vectors so the
        # shared plane profiles stay tight across cores
        qp, pp = d["qpat"], d["ppat"]
        o = np.lexsort((pp[:, 2], pp[:, 1], pp[:, 0],
                        qp[:, 2], qp[:, 1], qp[:, 0], cls))
        pos0cls = np.searchsorted(cls[o], np.arange(len(upat0)))
        rank = np.empty(len(halo), np.int64)
        rank[o] = np.arange(len(halo)) - pos0cls[cls[o]]
        slot_of_halo = starts0[cls] + rank             # halo idx -> slot
        slot_node = np.full(T0 * P, -1, np.int64)
        slot_node[slot_of_halo] = halo

        # idx0 from the descriptor list; higher-plane descs rank first
        e_slot = slot_of_halo[d["d_i"]]
        e_w, e_loc, e_np = d["d_w"], d["d_loc"], d["d_np"]
        oo = np.lexsort((-e_np, e_w, e_slot))
        so_s, so_w = e_slot[oo], e_w[oo]
        change = np.ones(len(oo), bool)
        change[1:] = (so_s[1:] != so_s[:-1]) | (so_w[1:] != so_w[:-1])
        startsr = np.nonzero(change)[0]
        gid = np.cumsum(change) - 1
        jj = np.empty(len(oo), np.int64)
        jj[oo] = np.arange(len(oo)) - startsr[gid]
        flat0, idx0 = _fill_idx(e_slot, e_loc, e_w, jj, colbase0, D0t,
                                C0, NW0)

        # per-tile quad/pair column profiles (this core)
        qc_slot = np.zeros((T0 * P, 3), np.int64)
        qc_slot[slot_of_halo] = d["qpat"]
        Q1_cores.append(qc_slot.reshape(T0, P, 3).max(axis=1))
        pc_slot = np.zeros((T0 * P, 3), np.int64)
        pc_slot[slot_of_halo] = d["qpat"] + d["ppat"]
        P1_cores.append(pc_slot.reshape(T0, P, 3).max(axis=1))

        # xh row table (fp8): quad rows hold 4 sources, pair rows 2,
        # single rows 1 (rest zero)
        uniq = d["uniq"]
        xh = np.zeros((NW0 * WSTR, 4 * IN), np.float32)
        rw, rl = d["row_w"], d["row_loc"]
        nq, npr = d["NQUAD"], d["NPAIR"]
        if nq:
            qrow = rw[:nq] * WSTR + rl[:nq]
            xh[qrow, :IN] = xhat[uniq[d["prow_a"][d["quad_p1"]]]]
            xh[qrow, IN:2 * IN] = xhat[uniq[d["prow_b"][d["quad_p1"]]]]
            xh[qrow, 2 * IN:3 * IN] = xhat[uniq[d["prow_a"][d["quad_p2"]]]]
            xh[qrow, 3 * IN:] = xhat[uniq[d["prow_b"][d["quad_p2"]]]]
        if npr:
            pids = np.nonzero(d["pair_used"])[0]
            prow = rw[nq:nq + npr] * WSTR + rl[nq:nq + npr]
            xh[prow, :IN] = xhat[uniq[d["prow_a"][pids]]]
            xh[prow, IN:2 * IN] = xhat[uniq[d["prow_b"][pids]]]
        if len(d["srow_ids"]):
            srow = rw[nq + npr:] * WSTR + rl[nq + npr:]
            xh[srow, :IN] = xhat[uniq[d["srow_ids"]]]
        xh = xh.astype(FP8)

        # xselfT: [IN, T0*P] column s = xhat[node(s)]
        xselfT = np.zeros((IN, T0 * P), np.float32)
        v = slot_node >= 0
        xselfT[:, v] = xhat[slot_node[v]].T
        xselfT = xselfT.astype(FP8)

        # per-slot scales
        dis0 = np.zeros(T0 * P, np.float32)
        dis0[v] = (dis * dis)[slot_node[v]]
        inv0 = np.zeros(T0 * P, np.float32)
        inv0[v] = invd[slot_node[v]]

        # ---------------- layer-1 structure ----------------
        own = d["own"]
        pos_of_node = np.full(N, -1, np.int64)
        pos_of_node[slot_node[v]] = np.nonzero(v)[0]
        degc = indptr[own + 1] - indptr[own]
        E1 = int(degc.sum())
        rep1 = np.repeat(np.arange(SH), degc)
        ei1 = np.repeat(indptr[own], degc) + (
            np.arange(E1) - np.repeat(np.cumsum(degc) - degc, degc))
        l1src = s_sorted[ei1]
        # append self edges
        rep1 = np.concatenate([rep1, np.arange(SH)])
        l1src = np.concatenate([l1src, own])
        spos = pos_of_node[l1src]
        assert (spos >= 0).all()
        w1 = spos // WIN1
        loc1 = _h1row(spos) - w1 * WSTR
        pat1 = np.zeros((SH, 2), np.int64)
        np.add.at(pat1, (rep1, w1), 1)
        pats1.append(pat1)
        l1_parts.append(dict(rep1=rep1, w1=w1, loc1=loc1, pat1=pat1))

        cores.append(dict(slot_node=slot_node, idx0=idx0, flat0=flat0,
                          xh=xh, xselfT=xselfT, dis0=dis0, inv0=inv0))

    P1 = np.maximum.reduce(P1_cores)
    P1 = np.minimum(P1, D0)                    # 2-plane loop bound <= D
    P1t = tuple(tuple(int(v) for v in row) for row in P1)
    Q1 = np.maximum.reduce(Q1_cores)
    Q1 = np.minimum(Q1, P1)                    # 4-plane loop bound <= P1
    Q1t = tuple(tuple(int(v) for v in row) for row in Q1)

    NW1 = -(-(T0 * P) // WIN1)
    # layer-1 slots: per-core lexsort by pattern; shared profile = max
    T1 = -(-(-(-SH // P)) // 4) * 4
    profs1 = []
    orders1 = []
    for c in range(NCORES):
        pat1 = l1_parts[c]["pat1"]
        o = np.lexsort(pat1.T[::-1])
        orders1.append(o)
        padded = np.zeros((T1 * P, 2), np.int64)
        padded[T1 * P - SH:] = pat1[o]
        profs1.append(padded.reshape(T1, P, 2).max(axis=1))
    D1 = np.maximum.reduce(profs1)
    D1t = tuple(tuple(int(v) for v in row) for row in D1)
    _, calls1, colbase1, C1 = _schedule(D1t, NW1)

    for c in range(NCORES):
        d = l1_parts[c]
        own = core_l0[c]["own"]
        o = orders1[c]
        slot1_of_own = np.empty(SH, np.int64)
        slot1_of_own[o] = T1 * P - SH + np.arange(SH)
        slot1_node = np.full(T1 * P, -1, np.int64)
        slot1_node[slot1_of_own] = own

        e_slot = slot1_of_own[d["rep1"]]
        jj = _rank_within(e_slot, d["w1"])
        flat1, idx1 = _fill_idx(e_slot, d["loc1"], d["w1"], jj, colbase1,
                                D1t, C1, NW1)

        v1 = slot1_node >= 0
        dis1 = np.zeros(T1 * P, np.float32)
        dis1[v1] = dis[slot1_node[v1]]
        inv1 = np.zeros(T1 * P, np.float32)
        inv1[v1] = invd[slot1_node[v1]]

        # pooling matrix, partition-tiled: [P, T1*G]
        mctT = np.zeros((P, T1 * G), np.float32)
        sn = slot1_node.reshape(T1, P)
        for t in range(T1):
            vt = sn[t] >= 0
            mctT[vt, t * G:(t + 1) * G] = Mhat[:, sn[t][vt]].T
        # fp8 would quantize pooling weights ~4%; scale rows to use the
        # format's range better is unnecessary -- keep bf16
        cd = cores[c]
        cd.update(idx1=idx1, flat1=flat1,
                  dis0=cd["dis0"].reshape(T0, P).T.copy(),
                  dis1=dis1.reshape(T1, P).T.copy(),
                  inv0=cd["inv0"].reshape(1, T0 * P).astype(BF16),
                  inv1=inv1.reshape(1, T1 * P).astype(BF16),
                  mctT=mctT.astype(BF16), slot1_node=slot1_node)

    shared = dict(
        w0=np.ascontiguousarray(W0, np.float32).astype(BF16),
        w1=np.ascontiguousarray(W1, np.float32).reshape(2, P, HID).astype(BF16),
        b0r=np.ascontiguousarray(b0, np.float32).reshape(1, HID).astype(BF16),
        b1r=np.ascontiguousarray(b1, np.float32).reshape(1, HID).astype(BF16),
        ident=np.eye(P, dtype=np.float32).astype(BF16),
    )
    zero_bias = bool(np.all(np.asarray(b0) == 0) and np.all(np.asarray(b1) == 0))
    meta = dict(N=N, IN=IN, HID=HID, G=G, SH=SH, T0=T0, T1=T1,
                NW0=NW0, NW1=NW1, C0=C0, C1=C1, zero_bias=zero_bias,
                D0=D0t, D1=D1t, P1=P1t, Q1=Q1t)
    fin = dict(W2=np.asarray(W2, np.float32), b2=np.asarray(b2, np.float32))
    return meta, shared, cores, fin


# ---------------------------------------------------------------------------
# Pure-numpy emulation of the device program (validation / debugging)
# ---------------------------------------------------------------------------

def _emulate(meta, shared, cores, fin):
    T0, T1, HID, IN, G = (meta[k] for k in ("T0", "T1", "HID", "IN", "G"))
    NW0, NW1 = meta["NW0"], meta["NW1"]
    D0, D1, P1, Q1 = meta["D0"], meta["D1"], meta["P1"], meta["Q1"]
    _, calls0, colbase0, C0 = _schedule(D0, NW0, GCOLS0, CCAP0)
    _, calls1, colbase1, C1 = _schedule(D1, NW1)
    w0 = shared["w0"].astype(np.float32)
    w1 = shared["w1"].astype(np.float32).reshape(2 * P, HID)
    b0 = shared["b0r"].astype(np.float32)[0]
    b1 = shared["b1r"].astype(np.float32)[0]

    def col_windows(calls, C):
        cw = np.zeros(C, np.int64)
        for w, n, off in calls:
            cw[off:off + n] = w
        return cw

    cw0 = col_windows(calls0, C0)
    cw1 = col_windows(calls1, C1)

    Y = np.zeros((G, HID), np.float32)
    for cd in cores:
        xh = cd["xh"].astype(np.float32)
        rows = cw0.repeat(P) * WSTR + cd["flat0"]
        gat = xh[rows].reshape(C0, P, 4 * IN)
        u0 = cd["xselfT"].astype(np.float32).T.copy()   # [T0*P, IN]
        for t in range(T0):
            for w in range(NW0):
                cb = colbase0[t][w]
                for j in range(D0[t][w]):
                    npl = 4 if j < Q1[t][w] else (2 if j < P1[t][w] else 1)
                    for pl in range(npl):
                        u0[t * P:(t + 1) * P] += \
                            gat[cb + j][:, pl * IN:(pl + 1) * IN]
        u0 = u0.astype(BF16).astype(np.float32)
        inv0 = cd["inv0"].astype(np.float32)[0]
        pre = u0 @ w0 + inv0[:, None] * b0[None, :]
        d0 = cd["dis0"].T.reshape(-1)
        h1 = np.maximum(d0[:, None] * pre, 0).astype(FP8).astype(np.float32)
        # place h1 into windowed layout (grouped rows)
        h1w = np.zeros((NW1 * WSTR, HID), np.float32)
        h1w[_h1row(np.arange(T0 * P))] = h1

        rows1 = cw1.repeat(P) * WSTR + cd["flat1"]
        gat1 = h1w[rows1].reshape(C1, P, HID)
        u1 = np.zeros((T1 * P, HID), np.float32)
        for t in range(T1):
            for w in range(NW1):
                cb = colbase1[t][w]
                for j in range(D1[t][w]):
                    u1[t * P:(t + 1) * P] += gat1[cb + j]
        u1 = u1.astype(BF16).astype(np.float32)
        inv1 = cd["inv1"].astype(np.float32)[0]
        pre1 = u1 @ w1 + inv1[:, None] * b1[None, :]
        d1 = cd["dis1"].T.reshape(-1)
        h2 = np.maximum(d1[:, None] * pre1, 0).astype(BF16).astype(np.float32)
        mctT = cd["mctT"].astype(np.float32)
        for t in range(T1):
            Y += mctT[:, t * G:(t + 1) * G].T @ h2[t * P:(t + 1) * P]
    return Y @ fin["W2"] + fin["b2"]


# ---------------------------------------------------------------------------
# Bass device program
# ---------------------------------------------------------------------------

def _build(meta):
    import concourse.bass as bass
    import concourse.mybir as mybir
    import concourse.tile as tile
    from concourse import bacc, library_config
    from concourse.tile_rust import add_dep_helper

    F32, I16 = mybir.dt.float32, mybir.dt.int16
    BF = mybir.dt.bfloat16
    F8 = mybir.dt.float8e4
    RELU = mybir.ActivationFunctionType.Relu

    IN, HID, G = meta["IN"], meta["HID"], meta["G"]
    T0, T1 = meta["T0"], meta["T1"]
    NW0, NW1 = meta["NW0"], meta["NW1"]
    C0, C1 = meta["C0"], meta["C1"]
    D0, D1, P1, Q1 = meta["D0"], meta["D1"], meta["P1"], meta["Q1"]
    ZB = meta["zero_bias"]
    groups0, calls0, colbase0, _ = _schedule(D0, NW0, GCOLS0, CCAP0)
    groups1, calls1, colbase1, _ = _schedule(D1, NW1)
    c2c0 = _call_of_col(calls0)
    c2c1 = _call_of_col(calls1)

    def _max_calls_per_group(groups, colbase, D, NW, c2c):
        mx = 1
        for (t0, t1) in groups:
            ids = set()
            for tt in range(t0, t1):
                for w in range(NW):
                    for j in range(D[tt][w]):
                        ids.add(c2c[colbase[tt][w] + j][0])
            mx = max(mx, len(ids))
        return mx

    gbufs0 = _max_calls_per_group(groups0, colbase0, D0, NW0, c2c0) + 4
    gbufs1 = _max_calls_per_group(groups1, colbase1, D1, NW1, c2c1) + 4

    nc = bacc.Bacc("TRN2", target_bir_lowering=False, debug=False,
                   num_devices=NCORES)

    t_xh = nc.dram_tensor("xh", [NW0 * WSTR, 4 * IN], F8,
                          kind="ExternalInput")
    t_xst = nc.dram_tensor("xselfT", [IN, T0 * P], F8, kind="ExternalInput")
    t_idx0 = nc.dram_tensor("idx0", [P, C0 * 8], I16, kind="ExternalInput")
    t_idx1 = nc.dram_tensor("idx1", [P, C1 * 8], I16, kind="ExternalInput")
    t_dis0 = nc.dram_tensor("dis0", [P, T0], F32, kind="ExternalInput")
    t_dis1 = nc.dram_tensor("dis1", [P, T1], F32, kind="ExternalInput")
    t_inv0 = nc.dram_tensor("inv0", [1, T0 * P], BF, kind="ExternalInput")
    t_inv1 = nc.dram_tensor("inv1", [1, T1 * P], BF, kind="ExternalInput")
    t_w0 = nc.dram_tensor("w0", [IN, HID], BF, kind="ExternalInput")
    t_w1 = nc.dram_tensor("w1", [2, P, HID], BF, kind="ExternalInput")
    t_b0 = nc.dram_tensor("b0r", [1, HID], BF, kind="ExternalInput")
    t_b1 = nc.dram_tensor("b1r", [1, HID], BF, kind="ExternalInput")
    t_mct = nc.dram_tensor("mctT", [P, T1 * G], BF, kind="ExternalInput")
    t_id = nc.dram_tensor("ident", [P, P], BF, kind="ExternalInput")
    t_out = nc.dram_tensor("outp", [G, HID], F32, kind="ExternalOutput")

    with tile.TileContext(nc) as tc:
        with (
            tc.tile_pool(name="const", bufs=1) as cpool,
            tc.tile_pool(name="ut", bufs=4) as upool,
            tc.tile_pool(name="stage", bufs=4) as spool,
            tc.tile_pool(name="h2res", bufs=10) as h2pool,
            tc.tile_pool(name="aggps", bufs=2, space="PSUM") as apool,
            tc.tile_pool(name="agg1ps", bufs=2, space="PSUM") as a1pool,
            tc.tile_pool(name="preps", bufs=3, space="PSUM") as ppool,
            tc.tile_pool(name="outps", bufs=1, space="PSUM") as opool,
            tc.tile_pool(name="dram", bufs=1, space="DRAM") as dpool,
        ):
            lib = nc.gpsimd.load_library(library_config.mlp)

            def cload(t, shape, dt):
                s = cpool.tile(shape, dt, tag=t.name)
                nc.sync.dma_start(s[:], t[:])
                return s

            ident = cload(t_id, [P, P], BF)
            ident8 = cpool.tile([P, P], F8, tag="ident8")
            nc.vector.tensor_copy(ident8[:], ident[:])
            w0 = cload(t_w0, [IN, HID], BF)
            w1 = cpool.tile([P, 2, HID], BF, tag="w1")
            nc.sync.dma_start(w1[:], t_w1[:].rearrange("j p h -> p j h"))
            if not ZB:
                b0r = cload(t_b0, [1, HID], BF)
                b1r = cload(t_b1, [1, HID], BF)
            dis0 = cload(t_dis0, [P, T0], F32)
            dis1 = cload(t_dis1, [P, T1], F32)
            mt = cload(t_mct, [P, T1 * G], BF)

            h1h = dpool.tile([NW1 * WSTR, HID], F8)

            h1h_writes = []
            zt = spool.tile([P, HID], F8, tag="zrow")
            nc.vector.memset(zt[:], 0.0)
            for w in range(NW1):
                h1h_writes.append(nc.sync.dma_start(
                    h1h[w * WSTR:w * WSTR + 1, :], zt[0:1, :]))

            def slot_row(s):
                return (s // WIN1) * WSTR + 1 + (s % WIN1)

            def do_gathers(t_src_ap, idx_sb, calls, grp_calls, buf_pool,
                           ccap_elem, elem, deps, dt=BF):
                out = {}
                for ci in grp_calls:
                    w, ncols, off = calls[ci]
                    gt = buf_pool.tile([P, ccap_elem], dt, tag="g")
                    gi = nc.gpsimd.dma_gather(
                        gt[:, :ncols * elem].rearrange(
                            "p (j d) -> p j d", j=ncols),
                        t_src_ap(w),
                        idx_sb[:, off * 8:(off + ncols) * 8],
                        ncols * P, ncols * P, elem, single_packet=False)
                    add_dep_helper(gi.ins, lib.ins, True, "gather after lib")
                    dl = deps(w) if callable(deps) else deps
                    for d in dl:
                        add_dep_helper(gi.ins, d.ins, True, "gather after src")
                    out[ci] = gt
                return out

            def grp_call_ids(t0, t1, colbase, D, NW, c2c):
                ids = set()
                for tt in range(t0, t1):
                    for w in range(NW):
                        for j in range(D[tt][w]):
                            ids.add(c2c[colbase[tt][w] + j][0])
                return sorted(ids)

            # ---------------- Layer 0 ----------------
            with tc.tile_pool(name="idx0p", bufs=1) as ipool0, \
                 tc.tile_pool(name="g0", bufs=gbufs0) as gpool0, \
                 tc.tile_pool(name="xs0", bufs=3) as xspool:
                idx0 = ipool0.tile([P, C0 * 8], I16, tag="idx0")
                nc.sync.dma_start(idx0[:], t_idx0[:])

                bank = None
                uT4 = None
                stage_t = None
                for (t0g, t1g) in groups0:
                    ids = grp_call_ids(t0g, t1g, colbase0, D0, NW0, c2c0)
                    bufs = do_gathers(
                        lambda w: t_xh[w * WSTR:(w + 1) * WSTR, :],
                        idx0, calls0, ids, gpool0, CCAP0 * 4 * IN, 4 * IN,
                        [], dt=F8)
                    # dense self-loop chunk for this group's tiles
                    xst = xspool.tile([P, (t1g - t0g) * P], F8, tag="xs")
                    nc.sync.dma_start(xst[:],
                                      t_xst[:, t0g * P:t1g * P])
                    for t in range(t0g, t1g):
                        q = t % 4
                        if q == 0:
                            bank = apool.tile([P, 4 * P], mybir.dt.float32,
                                              tag="aggps", space="PSUM")
                        ups = bank[:, q * P:(q + 1) * P]
                        nd = 0
                        for w in range(NW0):
                            for j in range(D0[t][w]):
                                nd += (4 if j < Q1[t][w]
                                       else (2 if j < P1[t][w] else 1))
                        # self contribution: ups = I @ xselfT_t
                        nc.tensor.matmul(
                            ups, lhsT=ident8[:],
                            rhs=xst[:, (t - t0g) * P:(t - t0g + 1) * P],
                            start=True, stop=(nd == 0))
                        k = 0
                        for w in range(NW0):
                            for j in range(D0[t][w]):
                                ci, lc = c2c0[colbase0[t][w] + j]
                                gt = bufs[ci]
                                nplanes = (4 if j < Q1[t][w]
                                           else (2 if j < P1[t][w] else 1))
                                for pl in range(nplanes):
                                    nc.tensor.matmul(
                                        ups,
                                        lhsT=gt[:, lc * 4 * IN + pl * IN:
                                                lc * 4 * IN + (pl + 1) * IN],
                                        rhs=ident8[:], start=False,
                                        stop=(k == nd - 1))
                                    k += 1
                        if q == 3:
                            uT4 = upool.tile([P, 4 * P], BF, tag="ut")
                            nc.vector.tensor_copy(uT4[:], bank[:])
                        # defer pre/act until uT4 is ready
                        if q == 3:
                            for tq in range(t - 3, t + 1):
                                qq = tq % 4
                                pre = ppool.tile([P, HID], mybir.dt.float32,
                                                 tag="preps", space="PSUM")
                                if ZB:
                                    nc.tensor.matmul(
                                        pre[:],
                                        lhsT=uT4[:, qq * P:(qq + 1) * P],
                                        rhs=w0[:], start=True, stop=True)
                                else:
                                    sd = upool.tile([1, P], BF, tag="seed")
                                    nc.sync.dma_start(
                                        sd[:], t_inv0[:, tq * P:(tq + 1) * P])
                                    nc.tensor.matmul(pre[:], lhsT=sd[:],
                                                     rhs=b0r[:],
                                                     start=True, stop=False)
                                    nc.tensor.matmul(
                                        pre[:],
                                        lhsT=uT4[:, qq * P:(qq + 1) * P],
                                        rhs=w0[:], start=False, stop=True)
                                if qq == 0:
                                    stage_t = spool.tile([P, 4 * HID], F8,
                                                         tag="h1stage")
                                nc.scalar.activation(
                                    stage_t[:, qq * HID:(qq + 1) * HID],
                                    pre[:], RELU, bias=0.0,
                                    scale=dis0[:, tq:tq + 1])
                                if qq == 3:
                                    r0 = slot_row((tq - 3) * P)
                                    h1h_writes.append(nc.sync.dma_start(
                                        h1h[r0:r0 + 4 * P, :].rearrange(
                                            "(p j) h -> p j h", p=P),
                                        stage_t[:].rearrange(
                                            "p (j h) -> p j h", j=4)))

            # ---------------- Layer 1 ----------------
            # writes [0, NW1) are the zero rows; block write i covers slots
            # [512i, 512i+512) -> window 512i // WIN1
            h1h_deps = {
                w: [h1h_writes[w]] + [wr for i, wr in
                                      enumerate(h1h_writes[NW1:])
                                      if (512 * i) // WIN1 == w]
                for w in range(NW1)
            }

            with tc.tile_pool(name="idx1p", bufs=1) as ipool1, \
                 tc.tile_pool(name="g1", bufs=gbufs1) as gpool1:
                idx1 = ipool1.tile([P, C1 * 8], I16, tag="idx1")
                nc.sync.dma_start(idx1[:], t_idx1[:])

                bank = None
                h2_tiles = {}
                pooled = 0
                ops = opool.tile([G, HID], mybir.dt.float32,
                                 tag="outps", space="PSUM")

                def pool_flush(upto):
                    nonlocal pooled
                    while pooled < upto:
                        tp = pooled
                        ht = h2_tiles.pop(tp)
                        nc.tensor.matmul(
                            ops[:], lhsT=mt[:, tp * G:(tp + 1) * G],
                            rhs=ht[:, (tp % 2) * HID:(tp % 2 + 1) * HID],
                            start=(tp == 0), stop=(tp == T1 - 1))
                        pooled += 1

                for (t0g, t1g) in groups1:
                    ids = grp_call_ids(t0g, t1g, colbase1, D1, NW1, c2c1)
                    bufs = do_gathers(
                        lambda w: h1h[w * WSTR:(w + 1) * WSTR, :],
                        idx1, calls1, ids, gpool1, CCAP * HID, HID,
                        lambda w: h1h_deps[w], dt=F8)
                    for t in range(t0g, t1g):
                        q = t % 2
                        if q == 0:
                            bank = a1pool.tile([P, 4 * P], mybir.dt.float32,
                                               tag="agg1ps", space="PSUM")
                        u0ps = bank[:, q * HID:q * HID + P]
                        u1ps = bank[:, q * HID + P:(q + 1) * HID]
                        nd = sum(D1[t])
                        if nd == 0:
                            nc.tensor.matmul(u0ps, lhsT=ident[:],
                                             rhs=ident[:], start=True,
                                             stop=True)
                            nc.tensor.matmul(u1ps, lhsT=ident[:],
                                             rhs=ident[:], start=True,
                                             stop=True)
                        for half, ups in ((0, u0ps), (1, u1ps)):
                            k = 0
                            for w in range(NW1):
                                for j in range(D1[t][w]):
                                    ci, lc = c2c1[colbase1[t][w] + j]
                                    gt = bufs[ci]
                                    nc.tensor.matmul(
                                        ups,
                                        lhsT=gt[:, lc * HID + half * P:
                                                lc * HID + (half + 1) * P],
                                        rhs=ident8[:], start=(k == 0),
                                        stop=(k == nd - 1))
                                    k += 1
                        if q == 1:
                            uT4 = upool.tile([P, 4 * P], BF, tag="ut")
                            nc.vector.tensor_copy(uT4[:], bank[:])
                            for tq in (t - 1, t):
                                qq = tq % 2
                                pre = ppool.tile([P, HID], mybir.dt.float32,
                                                 tag="preps", space="PSUM")
                                if not ZB:
                                    sd = upool.tile([1, P], BF, tag="seed")
                                    nc.sync.dma_start(
                                        sd[:], t_inv1[:, tq * P:(tq + 1) * P])
                                    nc.tensor.matmul(pre[:], lhsT=sd[:],
                                                     rhs=b1r[:],
                                                     start=True, stop=False)
                                nc.tensor.matmul(
                                    pre[:],
                                    lhsT=uT4[:, qq * HID:qq * HID + P],
                                    rhs=w1[:, 0, :], start=ZB, stop=False)
                                nc.tensor.matmul(
                                    pre[:],
                                    lhsT=uT4[:, qq * HID + P:(qq + 1) * HID],
                                    rhs=w1[:, 1, :], start=False, stop=True)
                                if tq % 2 == 0:
                                    h2_t = h2pool.tile([P, 2 * HID], BF,
                                                       tag="h2s")
                                    h2_tiles[tq] = h2_tiles[tq + 1] = h2_t
                                nc.scalar.activation(
                                    h2_tiles[tq][:, (tq % 2) * HID:
                                                 (tq % 2 + 1) * HID],
                                    pre[:], RELU, bias=0.0,
                                    scale=dis1[:, tq:tq + 1])
                                # pooling matmuls lag 16 tiles behind so
                                # they never wait on a fresh activation
                                pool_flush(max(0, tq - 15))

                pool_flush(T1)
                osb = spool.tile([G, HID], mybir.dt.float32, tag="osb")
                nc.vector.tensor_copy(osb[:], ops[:])
                nc.sync.dma_start(t_out[:], osb[:])

    nc.compile()
    return nc


# ---------------------------------------------------------------------------
# Entry point
# ---------------------------------------------------------------------------

_cache = {}


def _get_nc(meta):
    key = hashlib.sha1(repr(sorted(meta.items())).encode()).hexdigest()
    if key not in _cache:
        _cache[key] = _build(meta)
    return _cache[key]


def _in_maps(shared, cores):
    maps = []
    for cd in cores:
        m = dict(shared)
        m.update({k: cd[k] for k in
                  ("xh", "xselfT", "idx0", "idx1", "dis0", "dis1",
                   "inv0", "inv1", "mctT")})
        maps.append(m)
    return maps


def _run_device(meta, shared, cores):
    from concourse.bass_utils import run_bass_kernel_spmd
    nc = _get_nc(meta)
    res = run_bass_kernel_spmd(nc, _in_maps(shared, cores),
                               core_ids=list(range(NCORES)))
    return [r["outp"] for r in res.results]


def kernel(**inputs):
    meta, shared, cores, fin = _preprocess(**inputs)
    outs = _run_device(meta, shared, cores)
    Y = np.sum(np.stack(outs), axis=0, dtype=np.float32)
    out = Y @ fin["W2"] + fin["b2"]
    return out.astype(np.float32)


def profile_run(meta, shared, cores, trace_cores=None):
    """Profiled exec time in ns: NTFF trace when available, else the
    instruction-cost-model timeline simulation of the compiled program."""
    from concourse.bass_utils import run_bass_kernel_spmd
    nc = _get_nc(meta)
    try:
        res = run_bass_kernel_spmd(nc, _in_maps(shared, cores),
                                   core_ids=list(range(NCORES)), trace=True,
                                   trace_cores=trace_cores)
        if res.exec_time_ns is not None:
            print("profile:", res.instructions_and_trace[1]
                  if res.instructions_and_trace else None)
            return res.exec_time_ns
    except Exception as e:
        print(f"NTFF trace unavailable ({type(e).__name__}); "
              "using cost-model timeline")
    from concourse.timeline_sim import TimelineSim
    ts = TimelineSim(nc, trace=False)
    ts.simulate()
    return int(ts.time)
